# revision 39
# baseline (speedup 1.0000x reference)
"""NeRF attention Bass kernel for 8 Trainium2 NeuronCores.

Math (from the reference):
    pos = rays.reshape(N, 16),  f = features.reshape(N, 64),  N = 8192
    Q = LN(relu(pos@Wq1+bq1)@Wq2+bq2)*gq+bq_ln / 8           [N, 64]
    K = LN(relu(pos@Wk1+bk1)@Wk2+bk2)*gk+bk_ln               [N, 64]
    attn = softmax((Q @ K.T) * norm_scale, axis=-1)          [N, N]
    attn_out = attn @ f                                      [N, 64]
    returns (attn_out.reshape(8,32,32,64), attn)

Sharding: rows of Q / attn / attn_out are split across 8 cores (1024 rows
each); K, features and the tiny MLP params are replicated.

Device-side structure (per core, all feature-major / transposed):
  1. posT [17, n] built via PE transposes of rays tiles (ones bias row).
  2. MLPs: LayerNorm's centering and gain are LINEAR, so they are folded
     into W2 host-side:  t0g = W2cg.T @ relu(W1a.T @ posT_aug) is already
     centered-and-scaled; var comes from one PE matmul with a host-built
     1/(64*g^2) broadcast matrix; then rstd (ACT sqrt + DVE reciprocal)
     and out = t0g * rstdB + b.  norm_scale/8 is folded into Q's coeffs.
  3. phase B (token-major rows): scores tile = QT_blk.T @ KT chunk (PE,
     f32r) -> exp on ACT (accum_out emits row-sums for free) -> one DVE
     tensor_scalar normalize -> one 4MB row-block DMA out.
  4. phase C (key-major): scoresT = KT_tile.T @ QT (PE) -> exp (ACT) ->
     PV matmul accumulating attn_out.T over 64 key tiles; PE-transpose
     back to token-major and scale by the phase-B reciprocals.

Softmax skips the max-subtraction: scores are bounded (|s| < ~6 for this
problem family) so exp never overflows; this matches jax to ~1e-5.
Matmuls read fp32 data as float32r (fp22) which runs the PE at full rate.
"""

from contextlib import ExitStack

import numpy as np

import concourse.bass as bass
import concourse.mybir as mybir
import concourse.tile as tile
from concourse import bacc
from concourse.bass_utils import run_bass_kernel_spmd
from concourse.tile_rust import add_dep_helper

F32 = mybir.dt.float32
F32R = mybir.dt.float32r
AF = mybir.ActivationFunctionType
ALU = mybir.AluOpType
AX = mybir.AxisListType

N_CORES = 8
N = 8192          # total tokens (8*32*32)
NPOS = 16
NHID = 32
NE = 64           # embed dim == feature dim
SHARD = N // N_CORES          # 1024 rows per core
CHUNK = 512                   # free-dim chunk for matmuls / MLP
NCH = N // CHUNK              # 16 K-chunks
NCH_Q = SHARD // CHUNK        # 2 Q-chunks
NBLK = SHARD // 128           # 8 token blocks per core
NKT = N // 128                # 64 key tiles
EPS = 1e-5

_CACHE = {}


def _r(ap):
    """Read an fp32 AP as float32r (fp22 truncated, full-rate PE)."""
    return ap.bitcast(F32R)


def _build_program():
    nc = bacc.Bacc("TRN2", target_bir_lowering=False, debug=False)

    rays_all = nc.dram_tensor("rays_all", [N, NPOS], F32, kind="ExternalInput").ap()
    rays_sh = nc.dram_tensor("rays_sh", [SHARD, NPOS], F32, kind="ExternalInput").ap()
    feats = nc.dram_tensor("feats", [N, NE], F32R, kind="ExternalInput").ap()
    id128 = nc.dram_tensor("id128", [128, 128], F32, kind="ExternalInput").ap()
    # per-feature LN shift columns: bk_ln, bq_ln' (q pre-scaled by ns/8)
    lncoef = nc.dram_tensor("lncoef", [NE, 2], F32, kind="ExternalInput").ap()
    wk1a = nc.dram_tensor("wk1a", [NPOS + 1, NHID], F32R, kind="ExternalInput").ap()
    wk2c = nc.dram_tensor("wk2c", [NHID + 1, NE], F32R, kind="ExternalInput").ap()
    wq1a = nc.dram_tensor("wq1a", [NPOS + 1, NHID], F32R, kind="ExternalInput").ap()
    wq2c = nc.dram_tensor("wq2c", [NHID + 1, NE], F32R, kind="ExternalInput").ap()
    vark = nc.dram_tensor("vark", [NE, NE], F32R, kind="ExternalInput").ap()
    varq = nc.dram_tensor("varq", [NE, NE], F32R, kind="ExternalInput").ap()

    attn_w = nc.dram_tensor("attn_w", [SHARD, N], F32, kind="ExternalOutput").ap()
    attn_o = nc.dram_tensor("attn_o", [SHARD, NE], F32, kind="ExternalOutput").ap()

    with tile.TileContext(nc) as tc, ExitStack() as ctx:
        const = ctx.enter_context(tc.tile_pool(name="const", bufs=1))
        persist = ctx.enter_context(tc.tile_pool(name="persist", bufs=1))

        # ---------------- constants / inputs to SBUF ----------------
        ident = const.tile([128, 128], F32, tag="ident")
        nc.sync.dma_start(ident[:], id128)
        lnc = const.tile([NE, 2], F32, tag="lnc")
        nc.sync.dma_start(lnc[:], lncoef)
        w_k1 = const.tile([NPOS + 1, NHID], F32R, tag="wk1")
        nc.sync.dma_start(w_k1[:], wk1a)
        w_k2 = const.tile([NHID + 1, NE], F32R, tag="wk2")
        nc.sync.dma_start(w_k2[:], wk2c)
        w_q1 = const.tile([NPOS + 1, NHID], F32R, tag="wq1")
        nc.sync.dma_start(w_q1[:], wq1a)
        w_q2 = const.tile([NHID + 1, NE], F32R, tag="wq2")
        nc.sync.dma_start(w_q2[:], wq2c)
        v_k = const.tile([NE, NE], F32R, tag="vark")
        nc.sync.dma_start(v_k[:], vark)
        v_q = const.tile([NE, NE], F32R, tag="varq")
        nc.sync.dma_start(v_q[:], varq)
        eps_col = const.tile([128, 1], F32, tag="eps")
        nc.vector.memset(eps_col[:], EPS)

        # features, token-tiled: f_sb[:, kt*64:(kt+1)*64] = feats[kt*128:+128, :]
        f_sb = persist.tile([128, NKT * NE], F32R, tag="f")
        nc.sync.dma_start(
            f_sb[:].rearrange("p (t d) -> p t d", t=NKT),
            feats.rearrange("(t p) d -> p t d", p=128),
        )

        # ---------------- posT via PE transposes ----------------
        def build_posT(rays_ap, n_tok, tag, mlppool, psA):
            nt = n_tok // 128
            pos_sb = mlppool.tile([128, nt * NPOS], F32, tag=tag + "_tm",
                                  name=tag + "_tm")
            nc.sync.dma_start(
                pos_sb[:].rearrange("p (t j) -> p t j", t=nt),
                rays_ap.rearrange("(t p) j -> p t j", p=128),
            )
            posTa = mlppool.tile([NPOS + 1, n_tok], F32R, tag=tag, name=tag)
            # bias row (NPOS) must be ones; memset whole tile (gpsimd: DVE is
            # the MLP bottleneck), transposes overwrite rows 0-15
            nc.gpsimd.memset(posTa[:].bitcast(F32), 1.0)
            for g in range(0, nt, 4):
                tr = psA.tile([NPOS, 4 * 128], F32, tag="mlp", name="tr")
                for i in range(4):
                    t = g + i
                    nc.tensor.transpose(
                        tr[:, i * 128 : (i + 1) * 128],
                        pos_sb[:, t * NPOS : (t + 1) * NPOS],
                        ident[:],
                    )
                nc.vector.tensor_copy(posTa[0:NPOS, g * 128 : (g + 4) * 128], tr[:])
            return posTa

        # ---------------- feature-major MLP chunk ----------------
        def mlp_chunk(posTa, w1, w2c, vmat, b_col, c, h_sb, out_ap,
                      mlpsb, psA):
            """One [NE, CHUNK] chunk of the fused MLP+LayerNorm.

            w2c is the centered-and-gain-scaled second layer, so its matmul
            output t0g is (a - mean(a)) * g directly; vmat = 1/(64*g^2)
            broadcast matrix gives varB = var(a) replicated on every row.
            """
            cols = slice(c * CHUNK, (c + 1) * CHUNK)
            h_ps = psA.tile([NHID, CHUNK], F32, tag="mlp", name="h_ps")
            nc.tensor.matmul(h_ps[:], w1[:], _r(posTa[:, cols]))
            nc.vector.tensor_scalar(h_sb[0:NHID, :], h_ps[:], 0.0, None,
                                    op0=ALU.max)
            t0_ps = psA.tile([NE, CHUNK], F32, tag="mlp", name="t0_ps")
            nc.tensor.matmul(t0_ps[:], w2c[:], h_sb[:])
            t0_sb = mlpsb.tile([NE, CHUNK], F32R, tag="mlp_t0", name="t0_sb")
            nc.vector.tensor_copy(t0_sb[:], t0_ps[:])
            sqd = mlpsb.tile([NE, CHUNK], F32R, tag="mlp_sqd", name="sqd")
            nc.vector.tensor_tensor(
                sqd[:], t0_sb[:].bitcast(F32), t0_sb[:].bitcast(F32), op=ALU.mult
            )
            varB = psA.tile([NE, CHUNK], F32, tag="mlp", name="varB")
            nc.tensor.matmul(varB[:], vmat[:], sqd[:])
            # rstd = exp(-0.5 * ln(var + eps)): ln and exp share one ACT
            # table set (natural_log_exp_and_others), unlike sqrt, so the
            # MLP never forces a table swap against the softmax exps.
            lnv = mlpsb.tile([NE, CHUNK], F32, tag="mlp_sqB", name="lnv")
            nc.scalar.activation(lnv[:], varB[:], AF.Ln, bias=eps_col[0:NE, :])
            rstdB = mlpsb.tile([NE, CHUNK], F32, tag="mlp_rstd", name="rstdB")
            nc.scalar.activation(rstdB[:], lnv[:], AF.Exp, scale=-0.5)
            t1 = mlpsb.tile([NE, CHUNK], F32, tag="mlp_t1", name="t1")
            nc.vector.tensor_tensor(
                t1[:], t0_sb[:].bitcast(F32), rstdB[:], op=ALU.mult
            )
            nc.vector.tensor_scalar(out_ap, t1[:], b_col, None, op0=ALU.add)

        recip_all = persist.tile([128, NBLK], F32, tag="recip")

        # paired layout: kt_pair[p] rows 0-63 = K^T chunk 2p, rows 64-127 =
        # chunk 2p+1; qt_pair[t] duplicates Q^T chunk t in both halves.  This
        # lets scores matmuls run as row-packed pairs using both PE halves
        # (tile_position (0,0) + (64,0); column tiling is not supported).
        kt_pair = [persist.tile([128, CHUNK], F32R, tag=f"ktp{p}",
                                name=f"ktp{p}") for p in range(NCH // 2)]
        qt_pair = [persist.tile([128, CHUNK], F32R, tag=f"qtp{t}",
                                name=f"qtp{t}") for t in range(NCH_Q)]
        pv_sb = [persist.tile([NE, CHUNK], F32, tag=f"pvsb{t}",
                              name=f"pvsb{t}") for t in range(NCH_Q)]

        # ====== region 1: MLPs (keeps all ACT Sqrt before any Exp: the
        # sqrt and exp spline table sets are different and each switch costs
        # ~2.7us of ACT_TABLE_LOAD) ======
        with (
            tc.tile_pool(name="mlppool", bufs=1) as mlppool,
            tc.tile_pool(name="mlpsb", bufs=3) as mlpsb,
            tc.tile_pool(name="psA", bufs=8, space="PSUM") as psA,
        ):
            # rotating h tiles with the bias ones-row preset once
            h_tiles = []
            for i in range(3):
                h = mlppool.tile([NHID + 1, CHUNK], F32R, tag=f"hsb{i}",
                                 name=f"hsb{i}")
                nc.gpsimd.memset(h[NHID : NHID + 1, :].bitcast(F32), 1.0)
                h_tiles.append(h)

            posT_q = build_posT(rays_sh, SHARD, "posq", mlppool, psA)
            for t in range(NCH_Q):
                mlp_chunk(posT_q, w_q1, w_q2, v_q, lnc[:, 1:2], t,
                          h_tiles[t % 3], qt_pair[t][0:64, :], mlpsb, psA)
                nc.sync.dma_start(qt_pair[t][64:128, :], qt_pair[t][0:64, :])

            posT_k = build_posT(rays_all, N, "posk", mlppool, psA)
            for p in range(NCH // 2):
                mlp_chunk(posT_k, w_k1, w_k2, v_k, lnc[:, 0:1], 2 * p,
                          h_tiles[(2 * p) % 3], kt_pair[p][0:64, :],
                          mlpsb, psA)
                ktmp = mlppool.tile([NE, CHUNK], F32R, tag="ktmp",
                                    name="ktmp", bufs=2)
                mlp_chunk(posT_k, w_k1, w_k2, v_k, lnc[:, 0:1], 2 * p + 1,
                          h_tiles[(2 * p + 1) % 3], ktmp[:], mlpsb, psA)
                nc.sync.dma_start(kt_pair[p][64:128, :], ktmp[:])

        exppool = ctx.enter_context(tc.tile_pool(name="exppool", bufs=3))
        ctpool = ctx.enter_context(tc.tile_pool(name="ctpool", bufs=4))
        smpool = ctx.enter_context(tc.tile_pool(name="smpool", bufs=3))

        # ====== region 2: phase B (attn rows) + phase C (attn_out),
        # interleaved so ACT always has an exp ready while B is DMA-paced ==
        def phase_c_pair(p, psC, psP_pv):
            for tc_i in range(NCH_Q):
                for jj in range(4):
                    ka = p * 8 + jj
                    kb = p * 8 + 4 + jj
                    ct = psC.tile([128, 2 * CHUNK], F32, tag="ct", name="ct")
                    nc.tensor.matmul(
                        ct[:, 0:CHUNK],
                        kt_pair[p][0:64, jj * 128 : (jj + 1) * 128],
                        qt_pair[tc_i][0:64, :],
                    )
                    nc.tensor.matmul(
                        ct[:, CHUNK : 2 * CHUNK],
                        kt_pair[p][64:128, jj * 128 : (jj + 1) * 128],
                        qt_pair[tc_i][64:128, :],
                    )
                    expT = ctpool.tile([128, 2 * CHUNK], F32R, tag="expT",
                                       name="expT")
                    nc.scalar.activation(expT[:], ct[:], AF.Exp)
                    nc.tensor.matmul(
                        pv_ps[tc_i][:], f_sb[:, ka * NE : (ka + 1) * NE],
                        expT[:, 0:CHUNK],
                        start=(p == 0 and jj == 0), stop=False,
                    )
                    nc.tensor.matmul(
                        pv_ps[tc_i][:], f_sb[:, kb * NE : (kb + 1) * NE],
                        expT[:, CHUNK : 2 * CHUNK],
                        start=False,
                        stop=(p == NCH // 2 - 1 and jj == 3),
                    )

        def phase_b(b, psumB):
            tcq = b // 4
            bcols = slice((b % 4) * 128, (b % 4 + 1) * 128)
            exp_b = exppool.tile([128, N], F32, tag="exp", name="exp_b")
            part = smpool.tile([128, 4], F32, tag="part", name="part")
            for g in range(4):
                s_ps = psumB.tile([128, 4 * CHUNK], F32, tag="sps",
                                  name="s_ps")
                for j in range(2):
                    p = g * 2 + j
                    nc.tensor.matmul(
                        s_ps[:, (2 * j) * CHUNK : (2 * j + 1) * CHUNK],
                        qt_pair[tcq][0:64, bcols], kt_pair[p][0:64, :],
                    )
                    nc.tensor.matmul(
                        s_ps[:, (2 * j + 1) * CHUNK : (2 * j + 2) * CHUNK],
                        qt_pair[tcq][64:128, bcols], kt_pair[p][64:128, :],
                    )
                nc.scalar.activation(
                    exp_b[:, g * 4 * CHUNK : (g + 1) * 4 * CHUNK],
                    s_ps[:],
                    AF.Exp,
                    accum_out=part[:, g : g + 1],
                )
            rowsum = smpool.tile([128, 1], F32, tag="rs", name="rowsum")
            nc.vector.reduce_sum(rowsum[:], part[:], axis=AX.X)
            nc.vector.reciprocal(recip_all[:, b : b + 1], rowsum[:])
            nc.vector.tensor_scalar(
                exp_b[:], exp_b[:], recip_all[:, b : b + 1], None,
                op0=ALU.mult
            )
            nc.sync.dma_start(attn_w[b * 128 : (b + 1) * 128, :], exp_b[:])

        with (
            tc.tile_pool(name="psumB", bufs=1, space="PSUM") as psumB,
            tc.tile_pool(name="psC", bufs=1, space="PSUM") as psC,
            tc.tile_pool(name="psP", bufs=1, space="PSUM") as psP,
        ):
            pv_ps = [psP.tile([NE, CHUNK], F32, tag=f"pv{t}", name=f"pv{t}")
                     for t in range(NCH_Q)]
            for b in range(NBLK):
                phase_b(b, psumB)
                phase_c_pair(b, psC, psP)
            for t in range(NCH_Q):
                nc.vector.tensor_copy(pv_sb[t][:], pv_ps[t][:])

        # ====== tail: attn_out transpose + normalize ======
        with tc.tile_pool(name="psumT", bufs=2, space="PSUM") as psumT:
            for tc_i in range(NCH_Q):
                for i in range(CHUNK // 128):
                    b = tc_i * (CHUNK // 128) + i
                    tr = psumT.tile([128, NE], F32, tag="aotr", name="tr")
                    nc.tensor.transpose(
                        tr[:], pv_sb[tc_i][:, i * 128 : (i + 1) * 128],
                        ident[0:NE, 0:NE],
                    )
                    ao = smpool.tile([128, NE], F32, tag="ao", name="ao")
                    nc.vector.tensor_scalar(
                        ao[:], tr[:], recip_all[:, b : b + 1], None,
                        op0=ALU.mult
                    )
                    nc.sync.dma_start(attn_o[b * 128 : (b + 1) * 128, :], ao[:])

    nc.compile()
    return nc


def _prep_inputs(features, rays, scale, Wq1, bq1, Wq2, bq2, gq, bq_ln,
                 Wk1, bk1, Wk2, bk2, gk, bk_ln, Ws, bs):
    f32 = np.float32
    ns = (scale.astype(f32) @ Ws.astype(f32) + bs.astype(f32))[0]
    qscale = f32(ns) / f32(np.sqrt(f32(NE)))

    def aug(w, b):
        return np.ascontiguousarray(
            np.concatenate([w.astype(f32), b.astype(f32)[None, :]], axis=0))

    def centered(w2a, g):
        # fold LN centering + gain into the second layer: rows centered
        # over the output dim, then scaled per-output by g
        c = w2a - w2a.mean(axis=1, keepdims=True)
        return np.ascontiguousarray((c * g[None, :]).astype(f32))

    def varmat(g):
        # varB = vmat.T @ (t0*g)^2 with vmat[d',d] = 1/(64*g[d']^2)
        col = (1.0 / (NE * g.astype(np.float64) ** 2)).astype(f32)
        return np.ascontiguousarray(np.repeat(col[:, None], NE, axis=1))

    gq_s = gq.astype(f32) * qscale
    lncoef = np.ascontiguousarray(np.stack(
        [bk_ln.astype(f32), bq_ln.astype(f32) * qscale], axis=1
    ).astype(f32))

    rays2 = np.ascontiguousarray(rays.reshape(N, NPOS).astype(f32))
    common = {
        "rays_all": rays2,
        "feats": np.ascontiguousarray(features.reshape(N, NE).astype(f32)),
        "id128": np.eye(128, dtype=f32),
        "lncoef": lncoef,
        "wk1a": aug(Wk1, bk1),
        "wk2c": centered(aug(Wk2, bk2), gk.astype(f32)),
        "wq1a": aug(Wq1, bq1),
        "wq2c": centered(aug(Wq2, bq2), gq_s),
        "vark": varmat(gk.astype(f32)),
        "varq": varmat(gq_s),
    }
    in_maps = []
    for c in range(N_CORES):
        m = dict(common)
        m["rays_sh"] = np.ascontiguousarray(rays2[c * SHARD : (c + 1) * SHARD])
        in_maps.append(m)
    return in_maps


def kernel(**inputs):
    if "nc" not in _CACHE:
        _CACHE["nc"] = _build_program()
    nc = _CACHE["nc"]
    in_maps = _prep_inputs(**inputs)
    res = run_bass_kernel_spmd(nc, in_maps, core_ids=list(range(N_CORES)))
    attn_w = np.concatenate([res.results[c]["attn_w"] for c in range(N_CORES)], axis=0)
    attn_o = np.concatenate([res.results[c]["attn_o"] for c in range(N_CORES)], axis=0)
    seq, h, w = 8, 32, 32
    return attn_o.reshape(seq, h, w, NE), attn_w


# revision 40
# speedup vs baseline: 1.1529x; 1.1529x over previous
"""NeRF attention Bass kernel for 8 Trainium2 NeuronCores.

Math (from the reference):
    pos = rays.reshape(N, 16),  f = features.reshape(N, 64),  N = 8192
    Q = LN(relu(pos@Wq1+bq1)@Wq2+bq2)*gq+bq_ln / 8           [N, 64]
    K = LN(relu(pos@Wk1+bk1)@Wk2+bk2)*gk+bk_ln               [N, 64]
    attn = softmax((Q @ K.T) * norm_scale, axis=-1)          [N, N]
    attn_out = attn @ f                                      [N, 64]
    returns (attn_out.reshape(8,32,32,64), attn)

Sharding: rows of Q / attn / attn_out are split across 8 cores (1024 rows
each); K, features and the tiny MLP params are replicated.

Device-side structure (per core, all feature-major / transposed):
  1. posT [17, n] built via PE transposes of rays tiles (ones bias row).
  2. MLPs: LayerNorm's centering and gain are LINEAR, so they are folded
     into W2 host-side:  t0g = W2cg.T @ relu(W1a.T @ posT_aug) is already
     centered-and-scaled; var comes from one PE matmul with a host-built
     1/(64*g^2) broadcast matrix; then rstd (ACT sqrt + DVE reciprocal)
     and out = t0g * rstdB + b.  norm_scale/8 is folded into Q's coeffs.
  3. phase B (token-major rows): scores tile = QT_blk.T @ KT chunk (PE,
     f32r) -> exp on ACT (accum_out emits row-sums for free) -> one DVE
     tensor_scalar normalize -> one 4MB row-block DMA out.
  4. phase C (key-major): scoresT = KT_tile.T @ QT (PE) -> exp (ACT) ->
     PV matmul accumulating attn_out.T over 64 key tiles; PE-transpose
     back to token-major and scale by the phase-B reciprocals.

Softmax skips the max-subtraction: scores are bounded (|s| < ~6 for this
problem family) so exp never overflows; this matches jax to ~1e-5.
Matmuls read fp32 data as float32r (fp22) which runs the PE at full rate.
"""

from contextlib import ExitStack

import numpy as np

import concourse.bass as bass
import concourse.mybir as mybir
import concourse.tile as tile
from concourse import bacc
from concourse.bass_utils import run_bass_kernel_spmd
from concourse.tile_rust import add_dep_helper

F32 = mybir.dt.float32
F32R = mybir.dt.float32r
AF = mybir.ActivationFunctionType
ALU = mybir.AluOpType
AX = mybir.AxisListType

N_CORES = 8
N = 8192          # total tokens (8*32*32)
NPOS = 16
NHID = 32
NE = 64           # embed dim == feature dim
SHARD = N // N_CORES          # 1024 rows per core
CHUNK = 512                   # free-dim chunk for matmuls / MLP
NCH = N // CHUNK              # 16 K-chunks
NCH_Q = SHARD // CHUNK        # 2 Q-chunks
NBLK = SHARD // 128           # 8 token blocks per core
NKT = N // 128                # 64 key tiles
EPS = 1e-5

_CACHE = {}


def _r(ap):
    """Read an fp32 AP as float32r (fp22 truncated, full-rate PE)."""
    return ap.bitcast(F32R)


def _build_program():
    nc = bacc.Bacc("TRN2", target_bir_lowering=False, debug=False)

    rays_all = nc.dram_tensor("rays_all", [N, NPOS], F32, kind="ExternalInput").ap()
    rays_sh = nc.dram_tensor("rays_sh", [SHARD, NPOS], F32, kind="ExternalInput").ap()
    feats = nc.dram_tensor("feats", [N, NE], F32R, kind="ExternalInput").ap()
    id128 = nc.dram_tensor("id128", [128, 128], F32, kind="ExternalInput").ap()
    # per-feature LN shift columns: bk_ln, bq_ln' (q pre-scaled by ns/8)
    lncoef = nc.dram_tensor("lncoef", [NE, 2], F32, kind="ExternalInput").ap()
    wk1a = nc.dram_tensor("wk1a", [NPOS + 1, NHID], F32R, kind="ExternalInput").ap()
    wk2c = nc.dram_tensor("wk2c", [NHID + 1, NE], F32R, kind="ExternalInput").ap()
    wq1a = nc.dram_tensor("wq1a", [NPOS + 1, NHID], F32R, kind="ExternalInput").ap()
    wq2c = nc.dram_tensor("wq2c", [NHID + 1, NE], F32R, kind="ExternalInput").ap()
    vark = nc.dram_tensor("vark", [NE, NE], F32R, kind="ExternalInput").ap()
    varq = nc.dram_tensor("varq", [NE, NE], F32R, kind="ExternalInput").ap()

    attn_w = nc.dram_tensor("attn_w", [SHARD, N], F32, kind="ExternalOutput").ap()
    attn_o = nc.dram_tensor("attn_o", [SHARD, NE], F32, kind="ExternalOutput").ap()

    with tile.TileContext(nc) as tc, ExitStack() as ctx:
        const = ctx.enter_context(tc.tile_pool(name="const", bufs=1))
        persist = ctx.enter_context(tc.tile_pool(name="persist", bufs=1))

        # ---------------- constants / inputs to SBUF ----------------
        ident = const.tile([128, 128], F32, tag="ident")
        nc.sync.dma_start(ident[:], id128)
        lnc = const.tile([NE, 2], F32, tag="lnc")
        nc.sync.dma_start(lnc[:], lncoef)
        w_k1 = const.tile([NPOS + 1, NHID], F32R, tag="wk1")
        nc.sync.dma_start(w_k1[:], wk1a)
        w_k2 = const.tile([NHID + 1, NE], F32R, tag="wk2")
        nc.sync.dma_start(w_k2[:], wk2c)
        w_q1 = const.tile([NPOS + 1, NHID], F32R, tag="wq1")
        nc.sync.dma_start(w_q1[:], wq1a)
        w_q2 = const.tile([NHID + 1, NE], F32R, tag="wq2")
        nc.sync.dma_start(w_q2[:], wq2c)
        v_k = const.tile([NE, NE], F32R, tag="vark")
        nc.sync.dma_start(v_k[:], vark)
        v_q = const.tile([NE, NE], F32R, tag="varq")
        nc.sync.dma_start(v_q[:], varq)
        eps_col = const.tile([128, 1], F32, tag="eps")
        nc.vector.memset(eps_col[:], EPS)

        # features, token-tiled: f_sb[:, kt*64:(kt+1)*64] = feats[kt*128:+128, :]
        f_sb = persist.tile([128, NKT * NE], F32R, tag="f")
        nc.sync.dma_start(
            f_sb[:].rearrange("p (t d) -> p t d", t=NKT),
            feats.rearrange("(t p) d -> p t d", p=128),
        )

        # ---------------- posT via PE transposes ----------------
        def build_posT(rays_ap, n_tok, tag, mlppool, psA):
            nt = n_tok // 128
            pos_sb = mlppool.tile([128, nt * NPOS], F32, tag=tag + "_tm",
                                  name=tag + "_tm")
            nc.sync.dma_start(
                pos_sb[:].rearrange("p (t j) -> p t j", t=nt),
                rays_ap.rearrange("(t p) j -> p t j", p=128),
            )
            posTa = mlppool.tile([NPOS + 1, n_tok], F32R, tag=tag, name=tag)
            # bias row (NPOS) must be ones; memset whole tile (gpsimd: DVE is
            # the MLP bottleneck), transposes overwrite rows 0-15
            nc.gpsimd.memset(posTa[:].bitcast(F32), 1.0)
            for g in range(0, nt, 4):
                tr = psA.tile([NPOS, 4 * 128], F32, tag="mlp", name="tr")
                for i in range(4):
                    t = g + i
                    nc.tensor.transpose(
                        tr[:, i * 128 : (i + 1) * 128],
                        pos_sb[:, t * NPOS : (t + 1) * NPOS],
                        ident[:],
                    )
                nc.vector.tensor_copy(posTa[0:NPOS, g * 128 : (g + 4) * 128], tr[:])
            return posTa

        # ---------------- feature-major MLP chunk ----------------
        def mlp_chunk(posTa, w1, w2c, vmat, b_col, c, h_sb, out_ap,
                      mlpsb, psA):
            """One [NE, CHUNK] chunk of the fused MLP+LayerNorm.

            w2c is the centered-and-gain-scaled second layer, so its matmul
            output t0g is (a - mean(a)) * g directly; vmat = 1/(64*g^2)
            broadcast matrix gives varB = var(a) replicated on every row.
            """
            cols = slice(c * CHUNK, (c + 1) * CHUNK)
            h_ps = psA.tile([NHID, CHUNK], F32, tag="mlp", name="h_ps")
            nc.tensor.matmul(h_ps[:], w1[:], _r(posTa[:, cols]))
            nc.vector.tensor_scalar(h_sb[0:NHID, :], h_ps[:], 0.0, None,
                                    op0=ALU.max)
            t0_ps = psA.tile([NE, CHUNK], F32, tag="mlp", name="t0_ps")
            nc.tensor.matmul(t0_ps[:], w2c[:], h_sb[:])
            t0_sb = mlpsb.tile([NE, CHUNK], F32R, tag="mlp_t0", name="t0_sb")
            nc.vector.tensor_copy(t0_sb[:], t0_ps[:])
            sqd = mlpsb.tile([NE, CHUNK], F32R, tag="mlp_sqd", name="sqd")
            nc.vector.tensor_tensor(
                sqd[:], t0_sb[:].bitcast(F32), t0_sb[:].bitcast(F32), op=ALU.mult
            )
            varB = psA.tile([NE, CHUNK], F32, tag="mlp", name="varB")
            nc.tensor.matmul(varB[:], vmat[:], sqd[:])
            # rstd = exp(-0.5 * ln(var + eps)): ln and exp share one ACT
            # table set (natural_log_exp_and_others), unlike sqrt, so the
            # MLP never forces a table swap against the softmax exps.
            lnv = mlpsb.tile([NE, CHUNK], F32, tag="mlp_sqB", name="lnv")
            nc.scalar.activation(lnv[:], varB[:], AF.Ln, bias=eps_col[0:NE, :])
            rstdB = mlpsb.tile([NE, CHUNK], F32, tag="mlp_rstd", name="rstdB")
            nc.scalar.activation(rstdB[:], lnv[:], AF.Exp, scale=-0.5)
            t1 = mlpsb.tile([NE, CHUNK], F32, tag="mlp_t1", name="t1")
            nc.vector.tensor_tensor(
                t1[:], t0_sb[:].bitcast(F32), rstdB[:], op=ALU.mult
            )
            nc.vector.tensor_scalar(out_ap, t1[:], b_col, None, op0=ALU.add)

        recip_all = persist.tile([128, NBLK], F32, tag="recip")

        # paired layout: kt_pair[p] rows 0-63 = K^T chunk 2p, rows 64-127 =
        # chunk 2p+1; qt_pair[t] duplicates Q^T chunk t in both halves.  This
        # lets scores matmuls run as row-packed pairs using both PE halves
        # (tile_position (0,0) + (64,0); column tiling is not supported).
        kt_pair = [persist.tile([128, CHUNK], F32R, tag=f"ktp{p}",
                                name=f"ktp{p}") for p in range(NCH // 2)]
        qt_pair = [persist.tile([128, CHUNK], F32R, tag=f"qtp{t}",
                                name=f"qtp{t}") for t in range(NCH_Q)]
        pv_sb = [persist.tile([NE, CHUNK], F32, tag=f"pvsb{t}",
                              name=f"pvsb{t}") for t in range(NCH_Q)]

        # ====== region 1: MLPs (keeps all ACT Sqrt before any Exp: the
        # sqrt and exp spline table sets are different and each switch costs
        # ~2.7us of ACT_TABLE_LOAD) ======
        with (
            tc.tile_pool(name="mlppool", bufs=1) as mlppool,
            tc.tile_pool(name="mlpsb", bufs=3) as mlpsb,
            tc.tile_pool(name="psA", bufs=8, space="PSUM") as psA,
        ):
            # rotating h tiles with the bias ones-row preset once
            h_tiles = []
            for i in range(3):
                h = mlppool.tile([NHID + 1, CHUNK], F32R, tag=f"hsb{i}",
                                 name=f"hsb{i}")
                nc.gpsimd.memset(h[NHID : NHID + 1, :].bitcast(F32), 1.0)
                h_tiles.append(h)

            posT_q = build_posT(rays_sh, SHARD, "posq", mlppool, psA)
            for t in range(NCH_Q):
                mlp_chunk(posT_q, w_q1, w_q2, v_q, lnc[:, 1:2], t,
                          h_tiles[t % 3], qt_pair[t][0:64, :], mlpsb, psA)
                nc.sync.dma_start(qt_pair[t][64:128, :], qt_pair[t][0:64, :])

            posT_k = build_posT(rays_all, N, "posk", mlppool, psA)
            for p in range(NCH // 2):
                mlp_chunk(posT_k, w_k1, w_k2, v_k, lnc[:, 0:1], 2 * p,
                          h_tiles[(2 * p) % 3], kt_pair[p][0:64, :],
                          mlpsb, psA)
                ktmp = mlppool.tile([NE, CHUNK], F32R, tag="ktmp",
                                    name="ktmp", bufs=2)
                mlp_chunk(posT_k, w_k1, w_k2, v_k, lnc[:, 0:1], 2 * p + 1,
                          h_tiles[(2 * p + 1) % 3], ktmp[:], mlpsb, psA)
                nc.sync.dma_start(kt_pair[p][64:128, :], ktmp[:])

        exppool = ctx.enter_context(tc.tile_pool(name="exppool", bufs=3))
        ctpool = ctx.enter_context(tc.tile_pool(name="ctpool", bufs=4))
        smpool = ctx.enter_context(tc.tile_pool(name="smpool", bufs=3))

        # ====== region 2a: phase B (attn rows out), DMA-paced, fully
        # double-buffered scores groups ======
        def phase_b(b, psumB):
            tcq = b // 4
            bcols = slice((b % 4) * 128, (b % 4 + 1) * 128)
            exp_b = exppool.tile([128, N], F32, tag="exp", name="exp_b")
            part = smpool.tile([128, 4], F32, tag="part", name="part")
            for g in range(4):
                s_ps = psumB.tile([128, 4 * CHUNK], F32, tag="sps",
                                  name="s_ps")
                for j in range(2):
                    p = g * 2 + j
                    nc.tensor.matmul(
                        s_ps[:, (2 * j) * CHUNK : (2 * j + 1) * CHUNK],
                        qt_pair[tcq][0:64, bcols], kt_pair[p][0:64, :],
                    )
                    nc.tensor.matmul(
                        s_ps[:, (2 * j + 1) * CHUNK : (2 * j + 2) * CHUNK],
                        qt_pair[tcq][64:128, bcols], kt_pair[p][64:128, :],
                    )
                nc.scalar.activation(
                    exp_b[:, g * 4 * CHUNK : (g + 1) * 4 * CHUNK],
                    s_ps[:],
                    AF.Exp,
                    accum_out=part[:, g : g + 1],
                )
            rowsum = smpool.tile([128, 1], F32, tag="rs", name="rowsum")
            nc.vector.reduce_sum(rowsum[:], part[:], axis=AX.X)
            nc.vector.reciprocal(recip_all[:, b : b + 1], rowsum[:])
            for h in range(2):
                half = slice(h * (N // 2), (h + 1) * (N // 2))
                nc.vector.tensor_scalar(
                    exp_b[:, half], exp_b[:, half], recip_all[:, b : b + 1],
                    None, op0=ALU.mult
                )
                nc.sync.dma_start(attn_w[b * 128 : (b + 1) * 128, half],
                                  exp_b[:, half])

        with tc.tile_pool(name="psumB", bufs=2, space="PSUM") as psumB:
            for b in range(NBLK):
                phase_b(b, psumB)

        # ====== region 2b: phase C (attn_out via key-major exp) ======
        # processing order of key tiles: pair-packed (kt, kt+4) within each
        # kt_pair; exp batches span 3 key tiles (groups cross pair bounds)
        kt_seq = []
        for p in range(NCH // 2):
            for jj in range(4):
                kt_seq.append((p, 0, jj))   # key tile 8p+jj     (rows 0-63)
                kt_seq.append((p, 1, jj))   # key tile 8p+4+jj   (rows 64-127)

        GRP = 3
        with (
            tc.tile_pool(name="psC", bufs=2, space="PSUM") as psC,
            tc.tile_pool(name="psP", bufs=1, space="PSUM") as psP,
        ):
            pv_ps = [psP.tile([NE, CHUNK], F32, tag=f"pv{t}", name=f"pv{t}")
                     for t in range(NCH_Q)]
            for tc_i in range(NCH_Q):
                s = 0
                while s < len(kt_seq):
                    n_in = min(GRP, len(kt_seq) - s)
                    ct = psC.tile([128, GRP * CHUNK], F32, tag="ct", name="ct")
                    for i in range(n_in):
                        p, half, jj = kt_seq[s + i]
                        base = half * 64
                        nc.tensor.matmul(
                            ct[:, i * CHUNK : (i + 1) * CHUNK],
                            kt_pair[p][base : base + 64,
                                       jj * 128 : (jj + 1) * 128],
                            qt_pair[tc_i][base : base + 64, :],
                        )
                    expT = ctpool.tile([128, GRP * CHUNK], F32R, tag="expT",
                                       name="expT")
                    nc.scalar.activation(expT[:, 0 : n_in * CHUNK],
                                         ct[:, 0 : n_in * CHUNK], AF.Exp)
                    for i in range(n_in):
                        p, half, jj = kt_seq[s + i]
                        kt = p * 8 + half * 4 + jj
                        nc.tensor.matmul(
                            pv_ps[tc_i][:],
                            f_sb[:, kt * NE : (kt + 1) * NE],
                            expT[:, i * CHUNK : (i + 1) * CHUNK],
                            start=(s + i == 0),
                            stop=(s + i == len(kt_seq) - 1),
                        )
                    s += n_in
            for t in range(NCH_Q):
                nc.vector.tensor_copy(pv_sb[t][:], pv_ps[t][:])

        # ====== tail: attn_out transpose + normalize ======
        with tc.tile_pool(name="psumT", bufs=2, space="PSUM") as psumT:
            for tc_i in range(NCH_Q):
                for i in range(CHUNK // 128):
                    b = tc_i * (CHUNK // 128) + i
                    tr = psumT.tile([128, NE], F32, tag="aotr", name="tr")
                    nc.tensor.transpose(
                        tr[:], pv_sb[tc_i][:, i * 128 : (i + 1) * 128],
                        ident[0:NE, 0:NE],
                    )
                    ao = smpool.tile([128, NE], F32, tag="ao", name="ao")
                    nc.vector.tensor_scalar(
                        ao[:], tr[:], recip_all[:, b : b + 1], None,
                        op0=ALU.mult
                    )
                    nc.sync.dma_start(attn_o[b * 128 : (b + 1) * 128, :], ao[:])

    nc.compile()
    return nc


def _prep_inputs(features, rays, scale, Wq1, bq1, Wq2, bq2, gq, bq_ln,
                 Wk1, bk1, Wk2, bk2, gk, bk_ln, Ws, bs):
    f32 = np.float32
    ns = (scale.astype(f32) @ Ws.astype(f32) + bs.astype(f32))[0]
    qscale = f32(ns) / f32(np.sqrt(f32(NE)))

    def aug(w, b):
        return np.ascontiguousarray(
            np.concatenate([w.astype(f32), b.astype(f32)[None, :]], axis=0))

    def centered(w2a, g):
        # fold LN centering + gain into the second layer: rows centered
        # over the output dim, then scaled per-output by g
        c = w2a - w2a.mean(axis=1, keepdims=True)
        return np.ascontiguousarray((c * g[None, :]).astype(f32))

    def varmat(g):
        # varB = vmat.T @ (t0*g)^2 with vmat[d',d] = 1/(64*g[d']^2)
        col = (1.0 / (NE * g.astype(np.float64) ** 2)).astype(f32)
        return np.ascontiguousarray(np.repeat(col[:, None], NE, axis=1))

    gq_s = gq.astype(f32) * qscale
    lncoef = np.ascontiguousarray(np.stack(
        [bk_ln.astype(f32), bq_ln.astype(f32) * qscale], axis=1
    ).astype(f32))

    rays2 = np.ascontiguousarray(rays.reshape(N, NPOS).astype(f32))
    common = {
        "rays_all": rays2,
        "feats": np.ascontiguousarray(features.reshape(N, NE).astype(f32)),
        "id128": np.eye(128, dtype=f32),
        "lncoef": lncoef,
        "wk1a": aug(Wk1, bk1),
        "wk2c": centered(aug(Wk2, bk2), gk.astype(f32)),
        "wq1a": aug(Wq1, bq1),
        "wq2c": centered(aug(Wq2, bq2), gq_s),
        "vark": varmat(gk.astype(f32)),
        "varq": varmat(gq_s),
    }
    in_maps = []
    for c in range(N_CORES):
        m = dict(common)
        m["rays_sh"] = np.ascontiguousarray(rays2[c * SHARD : (c + 1) * SHARD])
        in_maps.append(m)
    return in_maps


def kernel(**inputs):
    if "nc" not in _CACHE:
        _CACHE["nc"] = _build_program()
    nc = _CACHE["nc"]
    in_maps = _prep_inputs(**inputs)
    res = run_bass_kernel_spmd(nc, in_maps, core_ids=list(range(N_CORES)))
    attn_w = np.concatenate([res.results[c]["attn_w"] for c in range(N_CORES)], axis=0)
    attn_o = np.concatenate([res.results[c]["attn_o"] for c in range(N_CORES)], axis=0)
    seq, h, w = 8, 32, 32
    return attn_o.reshape(seq, h, w, NE), attn_w


# revision 41
# speedup vs baseline: 1.1579x; 1.0043x over previous
"""NeRF attention Bass kernel for 8 Trainium2 NeuronCores.

Math (from the reference):
    pos = rays.reshape(N, 16),  f = features.reshape(N, 64),  N = 8192
    Q = LN(relu(pos@Wq1+bq1)@Wq2+bq2)*gq+bq_ln / 8           [N, 64]
    K = LN(relu(pos@Wk1+bk1)@Wk2+bk2)*gk+bk_ln               [N, 64]
    attn = softmax((Q @ K.T) * norm_scale, axis=-1)          [N, N]
    attn_out = attn @ f                                      [N, 64]
    returns (attn_out.reshape(8,32,32,64), attn)

Sharding: rows of Q / attn / attn_out are split across 8 cores (1024 rows
each); K, features and the tiny MLP params are replicated.

Device-side structure (per core, all feature-major / transposed):
  1. posT [17, n] built via PE transposes of rays tiles (ones bias row).
  2. MLPs: LayerNorm's centering and gain are LINEAR, so they are folded
     into W2 host-side:  t0g = W2cg.T @ relu(W1a.T @ posT_aug) is already
     centered-and-scaled; var comes from one PE matmul with a host-built
     1/(64*g^2) broadcast matrix; then rstd (ACT sqrt + DVE reciprocal)
     and out = t0g * rstdB + b.  norm_scale/8 is folded into Q's coeffs.
  3. phase B (token-major rows): scores tile = QT_blk.T @ KT chunk (PE,
     f32r) -> exp on ACT (accum_out emits row-sums for free) -> one DVE
     tensor_scalar normalize -> one 4MB row-block DMA out.
  4. phase C (key-major): scoresT = KT_tile.T @ QT (PE) -> exp (ACT) ->
     PV matmul accumulating attn_out.T over 64 key tiles; PE-transpose
     back to token-major and scale by the phase-B reciprocals.

Softmax skips the max-subtraction: scores are bounded (|s| < ~6 for this
problem family) so exp never overflows; this matches jax to ~1e-5.
Matmuls read fp32 data as float32r (fp22) which runs the PE at full rate.
"""

from contextlib import ExitStack

import numpy as np

import concourse.bass as bass
import concourse.mybir as mybir
import concourse.tile as tile
from concourse import bacc
from concourse.bass_utils import run_bass_kernel_spmd
from concourse.tile_rust import add_dep_helper

F32 = mybir.dt.float32
F32R = mybir.dt.float32r
AF = mybir.ActivationFunctionType
ALU = mybir.AluOpType
AX = mybir.AxisListType

N_CORES = 8
N = 8192          # total tokens (8*32*32)
NPOS = 16
NHID = 32
NE = 64           # embed dim == feature dim
SHARD = N // N_CORES          # 1024 rows per core
CHUNK = 512                   # free-dim chunk for matmuls / MLP
NCH = N // CHUNK              # 16 K-chunks
NCH_Q = SHARD // CHUNK        # 2 Q-chunks
NBLK = SHARD // 128           # 8 token blocks per core
NKT = N // 128                # 64 key tiles
EPS = 1e-5

_CACHE = {}


def _r(ap):
    """Read an fp32 AP as float32r (fp22 truncated, full-rate PE)."""
    return ap.bitcast(F32R)


def _build_program():
    nc = bacc.Bacc("TRN2", target_bir_lowering=False, debug=False)

    rays_all = nc.dram_tensor("rays_all", [N, NPOS], F32, kind="ExternalInput").ap()
    rays_sh = nc.dram_tensor("rays_sh", [SHARD, NPOS], F32, kind="ExternalInput").ap()
    feats = nc.dram_tensor("feats", [N, NE], F32R, kind="ExternalInput").ap()
    id128 = nc.dram_tensor("id128", [128, 128], F32, kind="ExternalInput").ap()
    # per-feature LN shift columns: bk_ln, bq_ln' (q pre-scaled by ns/8)
    lncoef = nc.dram_tensor("lncoef", [NE, 2], F32, kind="ExternalInput").ap()
    wk1a = nc.dram_tensor("wk1a", [NPOS + 1, NHID], F32R, kind="ExternalInput").ap()
    wk2c = nc.dram_tensor("wk2c", [NHID + 1, NE], F32R, kind="ExternalInput").ap()
    wq1a = nc.dram_tensor("wq1a", [NPOS + 1, NHID], F32R, kind="ExternalInput").ap()
    wq2c = nc.dram_tensor("wq2c", [NHID + 1, NE], F32R, kind="ExternalInput").ap()
    vark = nc.dram_tensor("vark", [NE, NE], F32R, kind="ExternalInput").ap()
    varq = nc.dram_tensor("varq", [NE, NE], F32R, kind="ExternalInput").ap()

    attn_w = nc.dram_tensor("attn_w", [SHARD, N], F32, kind="ExternalOutput").ap()
    attn_o = nc.dram_tensor("attn_o", [SHARD, NE], F32, kind="ExternalOutput").ap()

    with tile.TileContext(nc) as tc, ExitStack() as ctx:
        const = ctx.enter_context(tc.tile_pool(name="const", bufs=1))
        persist = ctx.enter_context(tc.tile_pool(name="persist", bufs=1))

        # ---------------- constants / inputs to SBUF ----------------
        ident = const.tile([128, 128], F32, tag="ident")
        nc.sync.dma_start(ident[:], id128)
        lnc = const.tile([NE, 2], F32, tag="lnc")
        nc.sync.dma_start(lnc[:], lncoef)
        w_k1 = const.tile([NPOS + 1, NHID], F32R, tag="wk1")
        nc.sync.dma_start(w_k1[:], wk1a)
        w_k2 = const.tile([NHID + 1, NE], F32R, tag="wk2")
        nc.sync.dma_start(w_k2[:], wk2c)
        w_q1 = const.tile([NPOS + 1, NHID], F32R, tag="wq1")
        nc.sync.dma_start(w_q1[:], wq1a)
        w_q2 = const.tile([NHID + 1, NE], F32R, tag="wq2")
        nc.sync.dma_start(w_q2[:], wq2c)
        v_k = const.tile([NE, NE], F32R, tag="vark")
        nc.sync.dma_start(v_k[:], vark)
        v_q = const.tile([NE, NE], F32R, tag="varq")
        nc.sync.dma_start(v_q[:], varq)
        eps_col = const.tile([128, 1], F32, tag="eps")
        nc.vector.memset(eps_col[:], EPS)

        # features, token-tiled: f_sb[:, kt*64:(kt+1)*64] = feats[kt*128:+128, :]
        f_sb = persist.tile([128, NKT * NE], F32R, tag="f")
        nc.sync.dma_start(
            f_sb[:].rearrange("p (t d) -> p t d", t=NKT),
            feats.rearrange("(t p) d -> p t d", p=128),
        )

        # ---------------- posT via PE transposes ----------------
        def build_posT(rays_ap, n_tok, tag, mlppool, psA):
            nt = n_tok // 128
            pos_sb = mlppool.tile([128, nt * NPOS], F32, tag=tag + "_tm",
                                  name=tag + "_tm")
            nc.sync.dma_start(
                pos_sb[:].rearrange("p (t j) -> p t j", t=nt),
                rays_ap.rearrange("(t p) j -> p t j", p=128),
            )
            posTa = mlppool.tile([NPOS + 1, n_tok], F32R, tag=tag, name=tag)
            # bias row (NPOS) must be ones; memset whole tile (gpsimd: DVE is
            # the MLP bottleneck), transposes overwrite rows 0-15
            nc.gpsimd.memset(posTa[:].bitcast(F32), 1.0)
            for g in range(0, nt, 4):
                tr = psA.tile([NPOS, 4 * 128], F32, tag="mlp", name="tr")
                for i in range(4):
                    t = g + i
                    nc.tensor.transpose(
                        tr[:, i * 128 : (i + 1) * 128],
                        pos_sb[:, t * NPOS : (t + 1) * NPOS],
                        ident[:],
                    )
                nc.vector.tensor_copy(posTa[0:NPOS, g * 128 : (g + 4) * 128], tr[:])
            return posTa

        # ---------------- feature-major MLP chunk ----------------
        def mlp_chunk(posTa, w1, w2c, vmat, b_col, c, h_sb, out_ap,
                      mlpsb, psA):
            """One [NE, CHUNK] chunk of the fused MLP+LayerNorm.

            w2c is the centered-and-gain-scaled second layer, so its matmul
            output t0g is (a - mean(a)) * g directly; vmat = 1/(64*g^2)
            broadcast matrix gives varB = var(a) replicated on every row.
            """
            cols = slice(c * CHUNK, (c + 1) * CHUNK)
            h_ps = psA.tile([NHID, CHUNK], F32, tag="mlp", name="h_ps")
            nc.tensor.matmul(h_ps[:], w1[:], _r(posTa[:, cols]))
            nc.vector.tensor_scalar(h_sb[0:NHID, :], h_ps[:], 0.0, None,
                                    op0=ALU.max)
            t0_ps = psA.tile([NE, CHUNK], F32, tag="mlp", name="t0_ps")
            nc.tensor.matmul(t0_ps[:], w2c[:], h_sb[:])
            t0_sb = mlpsb.tile([NE, CHUNK], F32R, tag="mlp_t0", name="t0_sb")
            nc.vector.tensor_copy(t0_sb[:], t0_ps[:])
            sqd = mlpsb.tile([NE, CHUNK], F32R, tag="mlp_sqd", name="sqd")
            nc.vector.tensor_tensor(
                sqd[:], t0_sb[:].bitcast(F32), t0_sb[:].bitcast(F32), op=ALU.mult
            )
            varB = psA.tile([NE, CHUNK], F32, tag="mlp", name="varB")
            nc.tensor.matmul(varB[:], vmat[:], sqd[:])
            # rstd = 1/sqrt(var + eps).  Sqrt keeps the MLP on a single ACT
            # table set (the region-2 exps are contiguous afterwards, so the
            # whole kernel pays only two ACT_TABLE_LOADs).
            sqB = mlpsb.tile([NE, CHUNK], F32, tag="mlp_sqB", name="sqB")
            nc.scalar.activation(sqB[:], varB[:], AF.Sqrt, bias=eps_col[0:NE, :])
            rstdB = mlpsb.tile([NE, CHUNK], F32, tag="mlp_rstd", name="rstdB")
            nc.vector.reciprocal_approx_fast(rstdB[:], sqB[:])
            t1 = mlpsb.tile([NE, CHUNK], F32, tag="mlp_t1", name="t1")
            nc.vector.tensor_tensor(
                t1[:], t0_sb[:].bitcast(F32), rstdB[:], op=ALU.mult
            )
            nc.vector.tensor_scalar(out_ap, t1[:], b_col, None, op0=ALU.add)

        recip_all = persist.tile([128, NBLK], F32, tag="recip")

        # paired layout: kt_pair[p] rows 0-63 = K^T chunk 2p, rows 64-127 =
        # chunk 2p+1; qt_pair[t] duplicates Q^T chunk t in both halves.  This
        # lets scores matmuls run as row-packed pairs using both PE halves
        # (tile_position (0,0) + (64,0); column tiling is not supported).
        kt_pair = [persist.tile([128, CHUNK], F32R, tag=f"ktp{p}",
                                name=f"ktp{p}") for p in range(NCH // 2)]
        qt_pair = [persist.tile([128, CHUNK], F32R, tag=f"qtp{t}",
                                name=f"qtp{t}") for t in range(NCH_Q)]
        pv_sb = [persist.tile([NE, CHUNK], F32, tag=f"pvsb{t}",
                              name=f"pvsb{t}") for t in range(NCH_Q)]

        # ====== region 1: MLPs (keeps all ACT Sqrt before any Exp: the
        # sqrt and exp spline table sets are different and each switch costs
        # ~2.7us of ACT_TABLE_LOAD) ======
        with (
            tc.tile_pool(name="mlppool", bufs=1) as mlppool,
            tc.tile_pool(name="mlpsb", bufs=3) as mlpsb,
            tc.tile_pool(name="psA", bufs=8, space="PSUM") as psA,
        ):
            # rotating h tiles with the bias ones-row preset once
            h_tiles = []
            for i in range(3):
                h = mlppool.tile([NHID + 1, CHUNK], F32R, tag=f"hsb{i}",
                                 name=f"hsb{i}")
                nc.gpsimd.memset(h[NHID : NHID + 1, :].bitcast(F32), 1.0)
                h_tiles.append(h)

            posT_q = build_posT(rays_sh, SHARD, "posq", mlppool, psA)
            for t in range(NCH_Q):
                mlp_chunk(posT_q, w_q1, w_q2, v_q, lnc[:, 1:2], t,
                          h_tiles[t % 3], qt_pair[t][0:64, :], mlpsb, psA)
                nc.sync.dma_start(qt_pair[t][64:128, :], qt_pair[t][0:64, :])

            posT_k = build_posT(rays_all, N, "posk", mlppool, psA)
            for p in range(NCH // 2):
                mlp_chunk(posT_k, w_k1, w_k2, v_k, lnc[:, 0:1], 2 * p,
                          h_tiles[(2 * p) % 3], kt_pair[p][0:64, :],
                          mlpsb, psA)
                ktmp = mlppool.tile([NE, CHUNK], F32R, tag="ktmp",
                                    name="ktmp", bufs=2)
                mlp_chunk(posT_k, w_k1, w_k2, v_k, lnc[:, 0:1], 2 * p + 1,
                          h_tiles[(2 * p + 1) % 3], ktmp[:], mlpsb, psA)
                nc.sync.dma_start(kt_pair[p][64:128, :], ktmp[:])

        exppool = ctx.enter_context(tc.tile_pool(name="exppool", bufs=3))
        ctpool = ctx.enter_context(tc.tile_pool(name="ctpool", bufs=4))
        smpool = ctx.enter_context(tc.tile_pool(name="smpool", bufs=3))

        # ====== region 2a: phase B (attn rows out), DMA-paced, fully
        # double-buffered scores groups ======
        def phase_b(b, psumB):
            tcq = b // 4
            bcols = slice((b % 4) * 128, (b % 4 + 1) * 128)
            exp_b = exppool.tile([128, N], F32, tag="exp", name="exp_b")
            part = smpool.tile([128, 4], F32, tag="part", name="part")
            for g in range(4):
                s_ps = psumB.tile([128, 4 * CHUNK], F32, tag="sps",
                                  name="s_ps")
                for j in range(2):
                    p = g * 2 + j
                    nc.tensor.matmul(
                        s_ps[:, (2 * j) * CHUNK : (2 * j + 1) * CHUNK],
                        qt_pair[tcq][0:64, bcols], kt_pair[p][0:64, :],
                    )
                    nc.tensor.matmul(
                        s_ps[:, (2 * j + 1) * CHUNK : (2 * j + 2) * CHUNK],
                        qt_pair[tcq][64:128, bcols], kt_pair[p][64:128, :],
                    )
                nc.scalar.activation(
                    exp_b[:, g * 4 * CHUNK : (g + 1) * 4 * CHUNK],
                    s_ps[:],
                    AF.Exp,
                    accum_out=part[:, g : g + 1],
                )
            rowsum = smpool.tile([128, 1], F32, tag="rs", name="rowsum")
            nc.vector.reduce_sum(rowsum[:], part[:], axis=AX.X)
            nc.vector.reciprocal(recip_all[:, b : b + 1], rowsum[:])
            for h in range(2):
                half = slice(h * (N // 2), (h + 1) * (N // 2))
                nc.vector.tensor_scalar(
                    exp_b[:, half], exp_b[:, half], recip_all[:, b : b + 1],
                    None, op0=ALU.mult
                )
                nc.sync.dma_start(attn_w[b * 128 : (b + 1) * 128, half],
                                  exp_b[:, half])

        with tc.tile_pool(name="psumB", bufs=2, space="PSUM") as psumB:
            for b in range(NBLK):
                phase_b(b, psumB)

        # ====== region 2b: phase C (attn_out via key-major exp) ======
        # processing order of key tiles: pair-packed (kt, kt+4) within each
        # kt_pair; exp batches span 3 key tiles (groups cross pair bounds)
        kt_seq = []
        for p in range(NCH // 2):
            for jj in range(4):
                kt_seq.append((p, 0, jj))   # key tile 8p+jj     (rows 0-63)
                kt_seq.append((p, 1, jj))   # key tile 8p+4+jj   (rows 64-127)

        GRP = 3
        with (
            tc.tile_pool(name="psC", bufs=2, space="PSUM") as psC,
            tc.tile_pool(name="psP", bufs=1, space="PSUM") as psP,
        ):
            pv_ps = [psP.tile([NE, CHUNK], F32, tag=f"pv{t}", name=f"pv{t}")
                     for t in range(NCH_Q)]
            for tc_i in range(NCH_Q):
                s = 0
                while s < len(kt_seq):
                    n_in = min(GRP, len(kt_seq) - s)
                    ct = psC.tile([128, GRP * CHUNK], F32, tag="ct", name="ct")
                    for i in range(n_in):
                        p, half, jj = kt_seq[s + i]
                        base = half * 64
                        nc.tensor.matmul(
                            ct[:, i * CHUNK : (i + 1) * CHUNK],
                            kt_pair[p][base : base + 64,
                                       jj * 128 : (jj + 1) * 128],
                            qt_pair[tc_i][base : base + 64, :],
                        )
                    expT = ctpool.tile([128, GRP * CHUNK], F32R, tag="expT",
                                       name="expT")
                    nc.scalar.activation(expT[:, 0 : n_in * CHUNK],
                                         ct[:, 0 : n_in * CHUNK], AF.Exp)
                    for i in range(n_in):
                        p, half, jj = kt_seq[s + i]
                        kt = p * 8 + half * 4 + jj
                        nc.tensor.matmul(
                            pv_ps[tc_i][:],
                            f_sb[:, kt * NE : (kt + 1) * NE],
                            expT[:, i * CHUNK : (i + 1) * CHUNK],
                            start=(s + i == 0),
                            stop=(s + i == len(kt_seq) - 1),
                        )
                    s += n_in
            for t in range(NCH_Q):
                nc.vector.tensor_copy(pv_sb[t][:], pv_ps[t][:])

        # ====== tail: attn_out transpose + normalize ======
        with tc.tile_pool(name="psumT", bufs=2, space="PSUM") as psumT:
            for tc_i in range(NCH_Q):
                for i in range(CHUNK // 128):
                    b = tc_i * (CHUNK // 128) + i
                    tr = psumT.tile([128, NE], F32, tag="aotr", name="tr")
                    nc.tensor.transpose(
                        tr[:], pv_sb[tc_i][:, i * 128 : (i + 1) * 128],
                        ident[0:NE, 0:NE],
                    )
                    ao = smpool.tile([128, NE], F32, tag="ao", name="ao")
                    nc.vector.tensor_scalar(
                        ao[:], tr[:], recip_all[:, b : b + 1], None,
                        op0=ALU.mult
                    )
                    nc.sync.dma_start(attn_o[b * 128 : (b + 1) * 128, :], ao[:])

    nc.compile()
    return nc


def _prep_inputs(features, rays, scale, Wq1, bq1, Wq2, bq2, gq, bq_ln,
                 Wk1, bk1, Wk2, bk2, gk, bk_ln, Ws, bs):
    f32 = np.float32
    ns = (scale.astype(f32) @ Ws.astype(f32) + bs.astype(f32))[0]
    qscale = f32(ns) / f32(np.sqrt(f32(NE)))

    def aug(w, b):
        return np.ascontiguousarray(
            np.concatenate([w.astype(f32), b.astype(f32)[None, :]], axis=0))

    def centered(w2a, g):
        # fold LN centering + gain into the second layer: rows centered
        # over the output dim, then scaled per-output by g
        c = w2a - w2a.mean(axis=1, keepdims=True)
        return np.ascontiguousarray((c * g[None, :]).astype(f32))

    def varmat(g):
        # varB = vmat.T @ (t0*g)^2 with vmat[d',d] = 1/(64*g[d']^2)
        col = (1.0 / (NE * g.astype(np.float64) ** 2)).astype(f32)
        return np.ascontiguousarray(np.repeat(col[:, None], NE, axis=1))

    gq_s = gq.astype(f32) * qscale
    lncoef = np.ascontiguousarray(np.stack(
        [bk_ln.astype(f32), bq_ln.astype(f32) * qscale], axis=1
    ).astype(f32))

    rays2 = np.ascontiguousarray(rays.reshape(N, NPOS).astype(f32))
    common = {
        "rays_all": rays2,
        "feats": np.ascontiguousarray(features.reshape(N, NE).astype(f32)),
        "id128": np.eye(128, dtype=f32),
        "lncoef": lncoef,
        "wk1a": aug(Wk1, bk1),
        "wk2c": centered(aug(Wk2, bk2), gk.astype(f32)),
        "wq1a": aug(Wq1, bq1),
        "wq2c": centered(aug(Wq2, bq2), gq_s),
        "vark": varmat(gk.astype(f32)),
        "varq": varmat(gq_s),
    }
    in_maps = []
    for c in range(N_CORES):
        m = dict(common)
        m["rays_sh"] = np.ascontiguousarray(rays2[c * SHARD : (c + 1) * SHARD])
        in_maps.append(m)
    return in_maps


def kernel(**inputs):
    if "nc" not in _CACHE:
        _CACHE["nc"] = _build_program()
    nc = _CACHE["nc"]
    in_maps = _prep_inputs(**inputs)
    res = run_bass_kernel_spmd(nc, in_maps, core_ids=list(range(N_CORES)))
    attn_w = np.concatenate([res.results[c]["attn_w"] for c in range(N_CORES)], axis=0)
    attn_o = np.concatenate([res.results[c]["attn_o"] for c in range(N_CORES)], axis=0)
    seq, h, w = 8, 32, 32
    return attn_o.reshape(seq, h, w, NE), attn_w


# revision 43
# speedup vs baseline: 1.2369x; 1.0683x over previous
"""NeRF attention Bass kernel for 8 Trainium2 NeuronCores.

Math (from the reference):
    pos = rays.reshape(N, 16),  f = features.reshape(N, 64),  N = 8192
    Q = LN(relu(pos@Wq1+bq1)@Wq2+bq2)*gq+bq_ln / 8           [N, 64]
    K = LN(relu(pos@Wk1+bk1)@Wk2+bk2)*gk+bk_ln               [N, 64]
    attn = softmax((Q @ K.T) * norm_scale, axis=-1)          [N, N]
    attn_out = attn @ f                                      [N, 64]
    returns (attn_out.reshape(8,32,32,64), attn)

Sharding: rows of Q / attn / attn_out are split across 8 cores (1024 rows
each); K, features and the tiny MLP params are replicated.

Device-side structure (per core, all feature-major / transposed):
  1. posT [17, n] built via PE transposes of rays tiles (ones bias row).
  2. MLPs: LayerNorm's centering and gain are LINEAR, so they are folded
     into W2 host-side:  t0g = W2cg.T @ relu(W1a.T @ posT_aug) is already
     centered-and-scaled; var comes from one PE matmul with a host-built
     1/(64*g^2) broadcast matrix; then rstd (ACT sqrt + DVE reciprocal)
     and out = t0g * rstdB + b.  norm_scale/8 is folded into Q's coeffs.
  3. phase B (token-major rows): scores tile = QT_blk.T @ KT chunk (PE,
     f32r) -> exp on ACT (accum_out emits row-sums for free) -> one DVE
     tensor_scalar normalize -> one 4MB row-block DMA out.
  4. phase C (key-major): scoresT = KT_tile.T @ QT (PE) -> exp (ACT) ->
     PV matmul accumulating attn_out.T over 64 key tiles; PE-transpose
     back to token-major and scale by the phase-B reciprocals.

Softmax skips the max-subtraction: scores are bounded (|s| < ~6 for this
problem family) so exp never overflows; this matches jax to ~1e-5.
Matmuls read fp32 data as float32r (fp22) which runs the PE at full rate.
"""

from contextlib import ExitStack

import numpy as np

import concourse.bass as bass
import concourse.mybir as mybir
import concourse.tile as tile
from concourse import bacc
from concourse.bass_utils import run_bass_kernel_spmd
from concourse.tile_rust import add_dep_helper

F32 = mybir.dt.float32
F32R = mybir.dt.float32r
AF = mybir.ActivationFunctionType
ALU = mybir.AluOpType
AX = mybir.AxisListType

N_CORES = 8
N = 8192          # total tokens (8*32*32)
NPOS = 16
NHID = 32
NE = 64           # embed dim == feature dim
SHARD = N // N_CORES          # 1024 rows per core
CHUNK = 512                   # free-dim chunk for matmuls / MLP
NCH = N // CHUNK              # 16 K-chunks
NCH_Q = SHARD // CHUNK        # 2 Q-chunks
NBLK = SHARD // 128           # 8 token blocks per core
NKT = N // 128                # 64 key tiles
EPS = 1e-5

_CACHE = {}


def _r(ap):
    """Read an fp32 AP as float32r (fp22 truncated, full-rate PE)."""
    return ap.bitcast(F32R)


def _build_program():
    nc = bacc.Bacc("TRN2", target_bir_lowering=False, debug=False)

    rays_all = nc.dram_tensor("rays_all", [N, NPOS], F32, kind="ExternalInput").ap()
    rays_sh = nc.dram_tensor("rays_sh", [SHARD, NPOS], F32, kind="ExternalInput").ap()
    feats = nc.dram_tensor("feats", [N, NE], F32R, kind="ExternalInput").ap()
    id128 = nc.dram_tensor("id128", [128, 128], F32, kind="ExternalInput").ap()
    # per-feature LN shift columns: bk_ln, bq_ln' (q pre-scaled by ns/8)
    lncoef = nc.dram_tensor("lncoef", [NE, 2], F32, kind="ExternalInput").ap()
    wk1a = nc.dram_tensor("wk1a", [NPOS + 1, NHID], F32R, kind="ExternalInput").ap()
    wk2c = nc.dram_tensor("wk2c", [NHID + 1, NE], F32R, kind="ExternalInput").ap()
    wq1a = nc.dram_tensor("wq1a", [NPOS + 1, NHID], F32R, kind="ExternalInput").ap()
    wq2c = nc.dram_tensor("wq2c", [NHID + 1, NE], F32R, kind="ExternalInput").ap()
    vark = nc.dram_tensor("vark", [NE, NE], F32R, kind="ExternalInput").ap()
    varq = nc.dram_tensor("varq", [NE, NE], F32R, kind="ExternalInput").ap()

    attn_w = nc.dram_tensor("attn_w", [SHARD, N], F32, kind="ExternalOutput").ap()
    attn_o = nc.dram_tensor("attn_o", [SHARD, NE], F32, kind="ExternalOutput").ap()

    with tile.TileContext(nc) as tc, ExitStack() as ctx:
        const = ctx.enter_context(tc.tile_pool(name="const", bufs=1))
        persist = ctx.enter_context(tc.tile_pool(name="persist", bufs=1))

        # ---------------- constants / inputs to SBUF ----------------
        ident = const.tile([128, 128], F32, tag="ident")
        nc.sync.dma_start(ident[:], id128)
        lnc = const.tile([NE, 2], F32, tag="lnc")
        nc.sync.dma_start(lnc[:], lncoef)
        w_k1 = const.tile([NPOS + 1, NHID], F32R, tag="wk1")
        nc.sync.dma_start(w_k1[:], wk1a)
        w_k2 = const.tile([NHID + 1, NE], F32R, tag="wk2")
        nc.sync.dma_start(w_k2[:], wk2c)
        w_q1 = const.tile([NPOS + 1, NHID], F32R, tag="wq1")
        nc.sync.dma_start(w_q1[:], wq1a)
        w_q2 = const.tile([NHID + 1, NE], F32R, tag="wq2")
        nc.sync.dma_start(w_q2[:], wq2c)
        v_k = const.tile([NE, NE], F32R, tag="vark")
        nc.sync.dma_start(v_k[:], vark)
        v_q = const.tile([NE, NE], F32R, tag="varq")
        nc.sync.dma_start(v_q[:], varq)
        eps_col = const.tile([128, 1], F32, tag="eps")
        nc.vector.memset(eps_col[:], EPS)

        # features, token-tiled: f_sb[:, kt*64:(kt+1)*64] = feats[kt*128:+128, :]
        f_sb = persist.tile([128, NKT * NE], F32R, tag="f")
        nc.sync.dma_start(
            f_sb[:].rearrange("p (t d) -> p t d", t=NKT),
            feats.rearrange("(t p) d -> p t d", p=128),
        )

        # ---------------- posT via PE transposes ----------------
        def build_posT(rays_ap, n_tok, tag, mlppool, psA):
            nt = n_tok // 128
            pos_sb = mlppool.tile([128, nt * NPOS], F32, tag=tag + "_tm",
                                  name=tag + "_tm")
            nc.sync.dma_start(
                pos_sb[:].rearrange("p (t j) -> p t j", t=nt),
                rays_ap.rearrange("(t p) j -> p t j", p=128),
            )
            posTa = mlppool.tile([NPOS + 1, n_tok], F32R, tag=tag, name=tag)
            # bias row (NPOS) must be ones; memset whole tile (gpsimd: DVE is
            # the MLP bottleneck), transposes overwrite rows 0-15
            nc.gpsimd.memset(posTa[:].bitcast(F32), 1.0)
            for g in range(0, nt, 4):
                tr = psA.tile([NPOS, 4 * 128], F32, tag="mlp", name="tr")
                for i in range(4):
                    t = g + i
                    nc.tensor.transpose(
                        tr[:, i * 128 : (i + 1) * 128],
                        pos_sb[:, t * NPOS : (t + 1) * NPOS],
                        ident[:],
                    )
                nc.vector.tensor_copy(posTa[0:NPOS, g * 128 : (g + 4) * 128], tr[:])
            return posTa

        # ---------------- feature-major MLP chunk (1024 wide) ----------------
        MCH = 2 * CHUNK
        def mlp_chunk(posTa, w1, w2c, vmat, b_col, c, h_sb, out_ap,
                      mlpsb, psA):
            """One [NE, CHUNK] chunk of the fused MLP+LayerNorm.

            w2c is the centered-and-gain-scaled second layer, so its matmul
            output t0g is (a - mean(a)) * g directly; vmat = 1/(64*g^2)
            broadcast matrix gives varB = var(a) replicated on every row.
            """
            h_ps = psA.tile([NHID, MCH], F32, tag="mlp", name="h_ps")
            for hh in range(2):
                cl = slice(c * MCH + hh * CHUNK, c * MCH + (hh + 1) * CHUNK)
                nc.tensor.matmul(h_ps[:, hh * CHUNK : (hh + 1) * CHUNK],
                                 w1[:], _r(posTa[:, cl]))
            nc.vector.tensor_scalar(h_sb[0:NHID, :], h_ps[:], 0.0, None,
                                    op0=ALU.max)
            t0_ps = psA.tile([NE, MCH], F32, tag="mlp", name="t0_ps")
            for hh in range(2):
                cc = slice(hh * CHUNK, (hh + 1) * CHUNK)
                nc.tensor.matmul(t0_ps[:, cc], w2c[:], h_sb[:, cc])
            t0_sb = mlpsb.tile([NE, MCH], F32R, tag="mlp_t0", name="t0_sb")
            nc.vector.tensor_copy(t0_sb[:], t0_ps[:])
            sqd = mlpsb.tile([NE, MCH], F32R, tag="mlp_sqd", name="sqd")
            nc.vector.tensor_tensor(
                sqd[:], t0_sb[:].bitcast(F32), t0_sb[:].bitcast(F32), op=ALU.mult
            )
            varB = psA.tile([NE, MCH], F32, tag="mlp", name="varB")
            for hh in range(2):
                cc = slice(hh * CHUNK, (hh + 1) * CHUNK)
                nc.tensor.matmul(varB[:, cc], vmat[:], sqd[:, cc])
            # rstd = 1/sqrt(var + eps).  Sqrt keeps the MLP on a single ACT
            # table set (the region-2 exps are contiguous afterwards, so the
            # whole kernel pays only two ACT_TABLE_LOADs).
            sqB = mlpsb.tile([NE, MCH], F32, tag="mlp_sqB", name="sqB")
            nc.scalar.activation(sqB[:], varB[:], AF.Sqrt, bias=eps_col[0:NE, :])
            rstdB = mlpsb.tile([NE, MCH], F32, tag="mlp_rstd", name="rstdB")
            nc.vector.reciprocal_approx_fast(rstdB[:], sqB[:])
            t1 = mlpsb.tile([NE, MCH], F32, tag="mlp_t1", name="t1")
            nc.vector.tensor_tensor(
                t1[:], t0_sb[:].bitcast(F32), rstdB[:], op=ALU.mult
            )
            nc.vector.tensor_scalar(out_ap, t1[:], b_col, None, op0=ALU.add)

        recip_all = persist.tile([128, NBLK], F32, tag="recip")

        # paired layout: kt_pair[p] rows 0-63 = K^T chunk 2p, rows 64-127 =
        # chunk 2p+1; qt_pair[t] duplicates Q^T chunk t in both halves.  This
        # lets scores matmuls run as row-packed pairs using both PE halves
        # (tile_position (0,0) + (64,0); column tiling is not supported).
        kt_pair = [persist.tile([128, CHUNK], F32R, tag=f"ktp{p}",
                                name=f"ktp{p}") for p in range(NCH // 2)]
        qt_pair = [persist.tile([128, CHUNK], F32R, tag=f"qtp{t}",
                                name=f"qtp{t}") for t in range(NCH_Q)]
        pv_sb = [persist.tile([NE, CHUNK], F32, tag=f"pvsb{t}",
                              name=f"pvsb{t}") for t in range(NCH_Q)]

        # ====== region 1: MLPs (keeps all ACT Sqrt before any Exp: the
        # sqrt and exp spline table sets are different and each switch costs
        # ~2.7us of ACT_TABLE_LOAD) ======
        with (
            tc.tile_pool(name="mlppool", bufs=1) as mlppool,
            tc.tile_pool(name="mlpsb", bufs=3) as mlpsb,
            tc.tile_pool(name="psA", bufs=4, space="PSUM") as psA,
        ):
            # rotating h tiles with the bias ones-row preset once
            h_tiles = []
            for i in range(3):
                h = mlppool.tile([NHID + 1, 2 * CHUNK], F32R, tag=f"hsb{i}",
                                 name=f"hsb{i}")
                nc.gpsimd.memset(h[NHID : NHID + 1, :].bitcast(F32), 1.0)
                h_tiles.append(h)

            posT_q = build_posT(rays_sh, SHARD, "posq", mlppool, psA)
            qtmp = mlppool.tile([NE, 2 * CHUNK], F32R, tag="qtmp", name="qtmp")
            mlp_chunk(posT_q, w_q1, w_q2, v_q, lnc[:, 1:2], 0,
                      h_tiles[0], qtmp[:], mlpsb, psA)
            for t in range(NCH_Q):
                half = slice(t * CHUNK, (t + 1) * CHUNK)
                nc.sync.dma_start(qt_pair[t][0:64, :], qtmp[:, half])
                nc.sync.dma_start(qt_pair[t][64:128, :], qtmp[:, half])

            posT_k = build_posT(rays_all, N, "posk", mlppool, psA)
            for p in range(NCH // 2):
                ktmp = mlppool.tile([NE, 2 * CHUNK], F32R, tag="ktmp",
                                    name="ktmp", bufs=2)
                mlp_chunk(posT_k, w_k1, w_k2, v_k, lnc[:, 0:1], p,
                          h_tiles[p % 3], ktmp[:], mlpsb, psA)
                nc.sync.dma_start(kt_pair[p][0:64, :], ktmp[:, 0:CHUNK])
                nc.sync.dma_start(kt_pair[p][64:128, :], ktmp[:, CHUNK:])

        exppool = ctx.enter_context(tc.tile_pool(name="exppool", bufs=3))
        ctpool = ctx.enter_context(tc.tile_pool(name="ctpool", bufs=4))
        smpool = ctx.enter_context(tc.tile_pool(name="smpool", bufs=3))

        # ====== region 2a: phase B (attn rows out), DMA-paced, fully
        # double-buffered scores groups ======
        def phase_b(b, psumB):
            tcq = b // 4
            bcols = slice((b % 4) * 128, (b % 4 + 1) * 128)
            exp_b = exppool.tile([128, N], F32, tag="exp", name="exp_b")
            part = smpool.tile([128, 4], F32, tag="part", name="part")
            for g in range(4):
                s_ps = psumB.tile([128, 4 * CHUNK], F32, tag="sps",
                                  name="s_ps")
                for j in range(2):
                    p = g * 2 + j
                    nc.tensor.matmul(
                        s_ps[:, (2 * j) * CHUNK : (2 * j + 1) * CHUNK],
                        qt_pair[tcq][0:64, bcols], kt_pair[p][0:64, :],
                    )
                    nc.tensor.matmul(
                        s_ps[:, (2 * j + 1) * CHUNK : (2 * j + 2) * CHUNK],
                        qt_pair[tcq][64:128, bcols], kt_pair[p][64:128, :],
                    )
                nc.scalar.activation(
                    exp_b[:, g * 4 * CHUNK : (g + 1) * 4 * CHUNK],
                    s_ps[:],
                    AF.Exp,
                    accum_out=part[:, g : g + 1],
                )
            rowsum = smpool.tile([128, 1], F32, tag="rs", name="rowsum")
            nc.vector.reduce_sum(rowsum[:], part[:], axis=AX.X)
            nc.vector.reciprocal(recip_all[:, b : b + 1], rowsum[:])
            for h in range(2):
                half = slice(h * (N // 2), (h + 1) * (N // 2))
                nc.vector.tensor_scalar(
                    exp_b[:, half], exp_b[:, half], recip_all[:, b : b + 1],
                    None, op0=ALU.mult
                )
                nc.sync.dma_start(attn_w[b * 128 : (b + 1) * 128, half],
                                  exp_b[:, half])

        with tc.tile_pool(name="psumB", bufs=2, space="PSUM") as psumB:
            for b in range(NBLK):
                phase_b(b, psumB)

        # ====== region 2b: phase C (attn_out via key-major exp) ======
        # processing order of key tiles: pair-packed (kt, kt+4) within each
        # kt_pair; exp batches span 3 key tiles (groups cross pair bounds)
        kt_seq = []
        for p in range(NCH // 2):
            for jj in range(4):
                kt_seq.append((p, 0, jj))   # key tile 8p+jj     (rows 0-63)
                kt_seq.append((p, 1, jj))   # key tile 8p+4+jj   (rows 64-127)

        GRP = 3
        with (
            tc.tile_pool(name="psC", bufs=2, space="PSUM") as psC,
            tc.tile_pool(name="psP", bufs=1, space="PSUM") as psP,
        ):
            pv_ps = [psP.tile([NE, CHUNK], F32, tag=f"pv{t}", name=f"pv{t}")
                     for t in range(NCH_Q)]
            for tc_i in range(NCH_Q):
                s = 0
                while s < len(kt_seq):
                    n_in = min(GRP, len(kt_seq) - s)
                    ct = psC.tile([128, GRP * CHUNK], F32, tag="ct", name="ct")
                    for i in range(n_in):
                        p, half, jj = kt_seq[s + i]
                        base = half * 64
                        nc.tensor.matmul(
                            ct[:, i * CHUNK : (i + 1) * CHUNK],
                            kt_pair[p][base : base + 64,
                                       jj * 128 : (jj + 1) * 128],
                            qt_pair[tc_i][base : base + 64, :],
                        )
                    expT = ctpool.tile([128, GRP * CHUNK], F32R, tag="expT",
                                       name="expT")
                    nc.scalar.activation(expT[:, 0 : n_in * CHUNK],
                                         ct[:, 0 : n_in * CHUNK], AF.Exp)
                    for i in range(n_in):
                        p, half, jj = kt_seq[s + i]
                        kt = p * 8 + half * 4 + jj
                        nc.tensor.matmul(
                            pv_ps[tc_i][:],
                            f_sb[:, kt * NE : (kt + 1) * NE],
                            expT[:, i * CHUNK : (i + 1) * CHUNK],
                            start=(s + i == 0),
                            stop=(s + i == len(kt_seq) - 1),
                        )
                    s += n_in
            for t in range(NCH_Q):
                nc.vector.tensor_copy(pv_sb[t][:], pv_ps[t][:])

        # ====== tail: attn_out transpose + normalize ======
        with tc.tile_pool(name="psumT", bufs=2, space="PSUM") as psumT:
            for tc_i in range(NCH_Q):
                for i in range(CHUNK // 128):
                    b = tc_i * (CHUNK // 128) + i
                    tr = psumT.tile([128, NE], F32, tag="aotr", name="tr")
                    nc.tensor.transpose(
                        tr[:], pv_sb[tc_i][:, i * 128 : (i + 1) * 128],
                        ident[0:NE, 0:NE],
                    )
                    ao = smpool.tile([128, NE], F32, tag="ao", name="ao")
                    nc.vector.tensor_scalar(
                        ao[:], tr[:], recip_all[:, b : b + 1], None,
                        op0=ALU.mult
                    )
                    nc.sync.dma_start(attn_o[b * 128 : (b + 1) * 128, :], ao[:])

    nc.compile()
    return nc


def _prep_inputs(features, rays, scale, Wq1, bq1, Wq2, bq2, gq, bq_ln,
                 Wk1, bk1, Wk2, bk2, gk, bk_ln, Ws, bs):
    f32 = np.float32
    ns = (scale.astype(f32) @ Ws.astype(f32) + bs.astype(f32))[0]
    qscale = f32(ns) / f32(np.sqrt(f32(NE)))

    def aug(w, b):
        return np.ascontiguousarray(
            np.concatenate([w.astype(f32), b.astype(f32)[None, :]], axis=0))

    def centered(w2a, g):
        # fold LN centering + gain into the second layer: rows centered
        # over the output dim, then scaled per-output by g
        c = w2a - w2a.mean(axis=1, keepdims=True)
        return np.ascontiguousarray((c * g[None, :]).astype(f32))

    def varmat(g):
        # varB = vmat.T @ (t0*g)^2 with vmat[d',d] = 1/(64*g[d']^2)
        col = (1.0 / (NE * g.astype(np.float64) ** 2)).astype(f32)
        return np.ascontiguousarray(np.repeat(col[:, None], NE, axis=1))

    gq_s = gq.astype(f32) * qscale
    lncoef = np.ascontiguousarray(np.stack(
        [bk_ln.astype(f32), bq_ln.astype(f32) * qscale], axis=1
    ).astype(f32))

    rays2 = np.ascontiguousarray(rays.reshape(N, NPOS).astype(f32))
    common = {
        "rays_all": rays2,
        "feats": np.ascontiguousarray(features.reshape(N, NE).astype(f32)),
        "id128": np.eye(128, dtype=f32),
        "lncoef": lncoef,
        "wk1a": aug(Wk1, bk1),
        "wk2c": centered(aug(Wk2, bk2), gk.astype(f32)),
        "wq1a": aug(Wq1, bq1),
        "wq2c": centered(aug(Wq2, bq2), gq_s),
        "vark": varmat(gk.astype(f32)),
        "varq": varmat(gq_s),
    }
    in_maps = []
    for c in range(N_CORES):
        m = dict(common)
        m["rays_sh"] = np.ascontiguousarray(rays2[c * SHARD : (c + 1) * SHARD])
        in_maps.append(m)
    return in_maps


def kernel(**inputs):
    if "nc" not in _CACHE:
        _CACHE["nc"] = _build_program()
    nc = _CACHE["nc"]
    in_maps = _prep_inputs(**inputs)
    res = run_bass_kernel_spmd(nc, in_maps, core_ids=list(range(N_CORES)))
    attn_w = np.concatenate([res.results[c]["attn_w"] for c in range(N_CORES)], axis=0)
    attn_o = np.concatenate([res.results[c]["attn_o"] for c in range(N_CORES)], axis=0)
    seq, h, w = 8, 32, 32
    return attn_o.reshape(seq, h, w, NE), attn_w


# revision 44
# speedup vs baseline: 1.2386x; 1.0013x over previous
"""NeRF attention Bass kernel for 8 Trainium2 NeuronCores.

Math (from the reference):
    pos = rays.reshape(N, 16),  f = features.reshape(N, 64),  N = 8192
    Q = LN(relu(pos@Wq1+bq1)@Wq2+bq2)*gq+bq_ln / 8           [N, 64]
    K = LN(relu(pos@Wk1+bk1)@Wk2+bk2)*gk+bk_ln               [N, 64]
    attn = softmax((Q @ K.T) * norm_scale, axis=-1)          [N, N]
    attn_out = attn @ f                                      [N, 64]
    returns (attn_out.reshape(8,32,32,64), attn)

Sharding: rows of Q / attn / attn_out are split across 8 cores (1024 rows
each); K, features and the tiny MLP params are replicated.

Device-side structure (per core, all feature-major / transposed):
  1. posT [17, n] built via PE transposes of rays tiles (ones bias row).
  2. MLPs: LayerNorm's centering and gain are LINEAR, so they are folded
     into W2 host-side:  t0g = W2cg.T @ relu(W1a.T @ posT_aug) is already
     centered-and-scaled; var comes from one PE matmul with a host-built
     1/(64*g^2) broadcast matrix; then rstd (ACT sqrt + DVE reciprocal)
     and out = t0g * rstdB + b.  norm_scale/8 is folded into Q's coeffs.
  3. phase B (token-major rows): scores tile = QT_blk.T @ KT chunk (PE,
     f32r) -> exp on ACT (accum_out emits row-sums for free) -> one DVE
     tensor_scalar normalize -> one 4MB row-block DMA out.
  4. phase C (key-major): scoresT = KT_tile.T @ QT (PE) -> exp (ACT) ->
     PV matmul accumulating attn_out.T over 64 key tiles; PE-transpose
     back to token-major and scale by the phase-B reciprocals.

Softmax skips the max-subtraction: scores are bounded (|s| < ~6 for this
problem family) so exp never overflows; this matches jax to ~1e-5.
Matmuls read fp32 data as float32r (fp22) which runs the PE at full rate.
"""

from contextlib import ExitStack

import numpy as np

import concourse.bass as bass
import concourse.mybir as mybir
import concourse.tile as tile
from concourse import bacc
from concourse.bass_utils import run_bass_kernel_spmd
from concourse.tile_rust import add_dep_helper

F32 = mybir.dt.float32
F32R = mybir.dt.float32r
AF = mybir.ActivationFunctionType
ALU = mybir.AluOpType
AX = mybir.AxisListType

N_CORES = 8
N = 8192          # total tokens (8*32*32)
NPOS = 16
NHID = 32
NE = 64           # embed dim == feature dim
SHARD = N // N_CORES          # 1024 rows per core
CHUNK = 512                   # free-dim chunk for matmuls / MLP
NCH = N // CHUNK              # 16 K-chunks
NCH_Q = SHARD // CHUNK        # 2 Q-chunks
NBLK = SHARD // 128           # 8 token blocks per core
NKT = N // 128                # 64 key tiles
EPS = 1e-5

_CACHE = {}


def _r(ap):
    """Read an fp32 AP as float32r (fp22 truncated, full-rate PE)."""
    return ap.bitcast(F32R)


def _build_program():
    nc = bacc.Bacc("TRN2", target_bir_lowering=False, debug=False)

    rays_all = nc.dram_tensor("rays_all", [N, NPOS], F32, kind="ExternalInput").ap()
    rays_sh = nc.dram_tensor("rays_sh", [SHARD, NPOS], F32, kind="ExternalInput").ap()
    feats = nc.dram_tensor("feats", [N, NE], F32R, kind="ExternalInput").ap()
    id128 = nc.dram_tensor("id128", [128, 128], F32, kind="ExternalInput").ap()
    # per-feature LN shift columns: bk_ln, bq_ln' (q pre-scaled by ns/8)
    lncoef = nc.dram_tensor("lncoef", [NE, 2], F32, kind="ExternalInput").ap()
    wk1a = nc.dram_tensor("wk1a", [NPOS + 1, NHID], F32R, kind="ExternalInput").ap()
    wk2c = nc.dram_tensor("wk2c", [NHID + 1, NE], F32R, kind="ExternalInput").ap()
    wq1a = nc.dram_tensor("wq1a", [NPOS + 1, NHID], F32R, kind="ExternalInput").ap()
    wq2c = nc.dram_tensor("wq2c", [NHID + 1, NE], F32R, kind="ExternalInput").ap()
    vark = nc.dram_tensor("vark", [NE, NE], F32R, kind="ExternalInput").ap()
    varq = nc.dram_tensor("varq", [NE, NE], F32R, kind="ExternalInput").ap()

    attn_w = nc.dram_tensor("attn_w", [SHARD, N], F32, kind="ExternalOutput").ap()
    attn_o = nc.dram_tensor("attn_o", [SHARD, NE], F32, kind="ExternalOutput").ap()

    with tile.TileContext(nc) as tc, ExitStack() as ctx:
        const = ctx.enter_context(tc.tile_pool(name="const", bufs=1))
        persist = ctx.enter_context(tc.tile_pool(name="persist", bufs=1))

        # ---------------- constants / inputs to SBUF ----------------
        ident = const.tile([128, 128], F32, tag="ident")
        nc.sync.dma_start(ident[:], id128)
        lnc = const.tile([NE, 2], F32, tag="lnc")
        nc.sync.dma_start(lnc[:], lncoef)
        w_k1 = const.tile([NPOS + 1, NHID], F32R, tag="wk1")
        nc.sync.dma_start(w_k1[:], wk1a)
        w_k2 = const.tile([NHID + 1, NE], F32R, tag="wk2")
        nc.sync.dma_start(w_k2[:], wk2c)
        w_q1 = const.tile([NPOS + 1, NHID], F32R, tag="wq1")
        nc.sync.dma_start(w_q1[:], wq1a)
        w_q2 = const.tile([NHID + 1, NE], F32R, tag="wq2")
        nc.sync.dma_start(w_q2[:], wq2c)
        v_k = const.tile([NE, NE], F32R, tag="vark")
        nc.sync.dma_start(v_k[:], vark)
        v_q = const.tile([NE, NE], F32R, tag="varq")
        nc.sync.dma_start(v_q[:], varq)
        eps_col = const.tile([128, 1], F32, tag="eps")
        nc.vector.memset(eps_col[:], EPS)

        # features, token-tiled: f_sb[:, kt*64:(kt+1)*64] = feats[kt*128:+128, :]
        f_sb = persist.tile([128, NKT * NE], F32R, tag="f")
        nc.sync.dma_start(
            f_sb[:].rearrange("p (t d) -> p t d", t=NKT),
            feats.rearrange("(t p) d -> p t d", p=128),
        )

        # ---------------- posT via PE transposes ----------------
        def build_posT(rays_ap, n_tok, tag, mlppool, psA):
            nt = n_tok // 128
            pos_sb = mlppool.tile([128, nt * NPOS], F32, tag=tag + "_tm",
                                  name=tag + "_tm")
            nc.sync.dma_start(
                pos_sb[:].rearrange("p (t j) -> p t j", t=nt),
                rays_ap.rearrange("(t p) j -> p t j", p=128),
            )
            posTa = mlppool.tile([NPOS + 1, n_tok], F32R, tag=tag, name=tag)
            # bias row (NPOS) must be ones; memset whole tile (gpsimd: DVE is
            # the MLP bottleneck), transposes overwrite rows 0-15
            nc.gpsimd.memset(posTa[:].bitcast(F32), 1.0)
            for g in range(0, nt, 4):
                tr = psA.tile([NPOS, 4 * 128], F32, tag="mlp", name="tr")
                for i in range(4):
                    t = g + i
                    nc.tensor.transpose(
                        tr[:, i * 128 : (i + 1) * 128],
                        pos_sb[:, t * NPOS : (t + 1) * NPOS],
                        ident[:],
                    )
                nc.vector.tensor_copy(posTa[0:NPOS, g * 128 : (g + 4) * 128], tr[:])
            return posTa

        # ---------------- feature-major MLP chunk (1024 wide) ----------------
        MCH = 2 * CHUNK
        def mlp_chunk(posTa, w1, w2c, vmat, b_col, c, h_sb, out_ap,
                      mlpsb, psA):
            """One [NE, CHUNK] chunk of the fused MLP+LayerNorm.

            w2c is the centered-and-gain-scaled second layer, so its matmul
            output t0g is (a - mean(a)) * g directly; vmat = 1/(64*g^2)
            broadcast matrix gives varB = var(a) replicated on every row.
            """
            h_ps = psA.tile([NHID, MCH], F32, tag="mlp", name="h_ps")
            for hh in range(2):
                cl = slice(c * MCH + hh * CHUNK, c * MCH + (hh + 1) * CHUNK)
                nc.tensor.matmul(h_ps[:, hh * CHUNK : (hh + 1) * CHUNK],
                                 w1[:], _r(posTa[:, cl]))
            nc.vector.tensor_scalar(h_sb[0:NHID, :], h_ps[:], 0.0, None,
                                    op0=ALU.max)
            t0_ps = psA.tile([NE, MCH], F32, tag="mlp", name="t0_ps")
            for hh in range(2):
                cc = slice(hh * CHUNK, (hh + 1) * CHUNK)
                nc.tensor.matmul(t0_ps[:, cc], w2c[:], h_sb[:, cc])
            t0_sb = mlpsb.tile([NE, MCH], F32R, tag="mlp_t0", name="t0_sb")
            nc.vector.tensor_copy(t0_sb[:], t0_ps[:])
            sqd = mlpsb.tile([NE, MCH], F32R, tag="mlp_sqd", name="sqd")
            nc.vector.tensor_tensor(
                sqd[:], t0_sb[:].bitcast(F32), t0_sb[:].bitcast(F32), op=ALU.mult
            )
            varB = psA.tile([NE, MCH], F32, tag="mlp", name="varB")
            for hh in range(2):
                cc = slice(hh * CHUNK, (hh + 1) * CHUNK)
                nc.tensor.matmul(varB[:, cc], vmat[:], sqd[:, cc])
            # rstd = 1/sqrt(var + eps).  Sqrt keeps the MLP on a single ACT
            # table set (the region-2 exps are contiguous afterwards, so the
            # whole kernel pays only two ACT_TABLE_LOADs).
            sqB = mlpsb.tile([NE, MCH], F32, tag="mlp_sqB", name="sqB")
            nc.scalar.activation(sqB[:], varB[:], AF.Sqrt, bias=eps_col[0:NE, :])
            rstdB = mlpsb.tile([NE, MCH], F32, tag="mlp_rstd", name="rstdB")
            nc.vector.reciprocal_approx_fast(rstdB[:], sqB[:])
            t1 = mlpsb.tile([NE, MCH], F32, tag="mlp_t1", name="t1")
            nc.vector.tensor_tensor(
                t1[:], t0_sb[:].bitcast(F32), rstdB[:], op=ALU.mult
            )
            nc.vector.tensor_scalar(out_ap, t1[:], b_col, None, op0=ALU.add)

        recip_all = persist.tile([128, NBLK], F32, tag="recip")

        # paired layout: kt_pair[p] rows 0-63 = K^T chunk 2p, rows 64-127 =
        # chunk 2p+1; qt_pair[t] duplicates Q^T chunk t in both halves.  This
        # lets scores matmuls run as row-packed pairs using both PE halves
        # (tile_position (0,0) + (64,0); column tiling is not supported).
        kt_pair = [persist.tile([128, CHUNK], F32R, tag=f"ktp{p}",
                                name=f"ktp{p}") for p in range(NCH // 2)]
        qt_pair = [persist.tile([128, CHUNK], F32R, tag=f"qtp{t}",
                                name=f"qtp{t}") for t in range(NCH_Q)]
        pv_sb = [persist.tile([NE, CHUNK], F32, tag=f"pvsb{t}",
                              name=f"pvsb{t}") for t in range(NCH_Q)]

        # ====== region 1: MLPs (keeps all ACT Sqrt before any Exp: the
        # sqrt and exp spline table sets are different and each switch costs
        # ~2.7us of ACT_TABLE_LOAD) ======
        with (
            tc.tile_pool(name="mlppool", bufs=1) as mlppool,
            tc.tile_pool(name="mlpsb", bufs=3) as mlpsb,
            tc.tile_pool(name="psA", bufs=4, space="PSUM") as psA,
        ):
            # rotating h tiles with the bias ones-row preset once
            h_tiles = []
            for i in range(3):
                h = mlppool.tile([NHID + 1, 2 * CHUNK], F32R, tag=f"hsb{i}",
                                 name=f"hsb{i}")
                nc.gpsimd.memset(h[NHID : NHID + 1, :].bitcast(F32), 1.0)
                h_tiles.append(h)

            posT_q = build_posT(rays_sh, SHARD, "posq", mlppool, psA)
            qtmp = mlppool.tile([NE, 2 * CHUNK], F32R, tag="qtmp", name="qtmp")
            mlp_chunk(posT_q, w_q1, w_q2, v_q, lnc[:, 1:2], 0,
                      h_tiles[0], qtmp[:], mlpsb, psA)
            for t in range(NCH_Q):
                half = slice(t * CHUNK, (t + 1) * CHUNK)
                nc.sync.dma_start(qt_pair[t][0:64, :], qtmp[:, half])
                nc.sync.dma_start(qt_pair[t][64:128, :], qtmp[:, half])

            posT_k = build_posT(rays_all, N, "posk", mlppool, psA)
            for p in range(NCH // 2):
                ktmp = mlppool.tile([NE, 2 * CHUNK], F32R, tag="ktmp",
                                    name="ktmp", bufs=2)
                mlp_chunk(posT_k, w_k1, w_k2, v_k, lnc[:, 0:1], p,
                          h_tiles[p % 3], ktmp[:], mlpsb, psA)
                nc.sync.dma_start(kt_pair[p][0:64, :], ktmp[:, 0:CHUNK])
                nc.sync.dma_start(kt_pair[p][64:128, :], ktmp[:, CHUNK:])

        exppool = ctx.enter_context(tc.tile_pool(name="exppool", bufs=3))
        ctpool = ctx.enter_context(tc.tile_pool(name="ctpool", bufs=6))
        smpool = ctx.enter_context(tc.tile_pool(name="smpool", bufs=3))

        # ====== region 2a: phase B (attn rows out), DMA-paced, fully
        # double-buffered scores groups ======
        def phase_b(b, psumB):
            tcq = b // 4
            bcols = slice((b % 4) * 128, (b % 4 + 1) * 128)
            exp_b = exppool.tile([128, N], F32, tag="exp", name="exp_b")
            part = smpool.tile([128, 4], F32, tag="part", name="part")
            for g in range(4):
                s_ps = psumB.tile([128, 4 * CHUNK], F32, tag="sps",
                                  name="s_ps")
                for j in range(2):
                    p = g * 2 + j
                    nc.tensor.matmul(
                        s_ps[:, (2 * j) * CHUNK : (2 * j + 1) * CHUNK],
                        qt_pair[tcq][0:64, bcols], kt_pair[p][0:64, :],
                    )
                    nc.tensor.matmul(
                        s_ps[:, (2 * j + 1) * CHUNK : (2 * j + 2) * CHUNK],
                        qt_pair[tcq][64:128, bcols], kt_pair[p][64:128, :],
                    )
                nc.scalar.activation(
                    exp_b[:, g * 4 * CHUNK : (g + 1) * 4 * CHUNK],
                    s_ps[:],
                    AF.Exp,
                    accum_out=part[:, g : g + 1],
                )
            rowsum = smpool.tile([128, 1], F32, tag="rs", name="rowsum")
            nc.vector.reduce_sum(rowsum[:], part[:], axis=AX.X)
            nc.vector.reciprocal(recip_all[:, b : b + 1], rowsum[:])
            for h in range(2):
                half = slice(h * (N // 2), (h + 1) * (N // 2))
                nc.vector.tensor_scalar(
                    exp_b[:, half], exp_b[:, half], recip_all[:, b : b + 1],
                    None, op0=ALU.mult
                )
                nc.sync.dma_start(attn_w[b * 128 : (b + 1) * 128, half],
                                  exp_b[:, half])

        with tc.tile_pool(name="psumB", bufs=2, space="PSUM") as psumB:
            for b in range(NBLK):
                phase_b(b, psumB)

        # ====== region 2b: phase C (attn_out via key-major exp) ======
        # processing order of key tiles: pair-packed (kt, kt+4) within each
        # kt_pair; exp batches span 3 key tiles (groups cross pair bounds)
        kt_seq = []
        for p in range(NCH // 2):
            for jj in range(4):
                kt_seq.append((p, 0, jj))   # key tile 8p+jj     (rows 0-63)
                kt_seq.append((p, 1, jj))   # key tile 8p+4+jj   (rows 64-127)

        GRP = 3
        with (
            tc.tile_pool(name="psC", bufs=2, space="PSUM") as psC,
            tc.tile_pool(name="psP", bufs=1, space="PSUM") as psP,
        ):
            pv_ps = [psP.tile([NE, CHUNK], F32, tag=f"pv{t}", name=f"pv{t}")
                     for t in range(NCH_Q)]
            starts = []
            for s0 in range(0, len(kt_seq), GRP):
                for t in range(NCH_Q):
                    starts.append((t, s0))
            for tc_i, s in starts:
                if True:
                    n_in = min(GRP, len(kt_seq) - s)
                    ct = psC.tile([128, GRP * CHUNK], F32, tag="ct", name="ct")
                    for i in range(n_in):
                        p, half, jj = kt_seq[s + i]
                        base = half * 64
                        nc.tensor.matmul(
                            ct[:, i * CHUNK : (i + 1) * CHUNK],
                            kt_pair[p][base : base + 64,
                                       jj * 128 : (jj + 1) * 128],
                            qt_pair[tc_i][base : base + 64, :],
                        )
                    expT = ctpool.tile([128, GRP * CHUNK], F32R, tag="expT",
                                       name="expT")
                    nc.scalar.activation(expT[:, 0 : n_in * CHUNK],
                                         ct[:, 0 : n_in * CHUNK], AF.Exp)
                    for i in range(n_in):
                        p, half, jj = kt_seq[s + i]
                        kt = p * 8 + half * 4 + jj
                        nc.tensor.matmul(
                            pv_ps[tc_i][:],
                            f_sb[:, kt * NE : (kt + 1) * NE],
                            expT[:, i * CHUNK : (i + 1) * CHUNK],
                            start=(s + i == 0),
                            stop=(s + i == len(kt_seq) - 1),
                        )
            for t in range(NCH_Q):
                nc.vector.tensor_copy(pv_sb[t][:], pv_ps[t][:])

        # ====== tail: attn_out transpose + normalize ======
        with tc.tile_pool(name="psumT", bufs=2, space="PSUM") as psumT:
            for tc_i in range(NCH_Q):
                for i in range(CHUNK // 128):
                    b = tc_i * (CHUNK // 128) + i
                    tr = psumT.tile([128, NE], F32, tag="aotr", name="tr")
                    nc.tensor.transpose(
                        tr[:], pv_sb[tc_i][:, i * 128 : (i + 1) * 128],
                        ident[0:NE, 0:NE],
                    )
                    ao = smpool.tile([128, NE], F32, tag="ao", name="ao")
                    nc.vector.tensor_scalar(
                        ao[:], tr[:], recip_all[:, b : b + 1], None,
                        op0=ALU.mult
                    )
                    nc.sync.dma_start(attn_o[b * 128 : (b + 1) * 128, :], ao[:])

    nc.compile()
    return nc


def _prep_inputs(features, rays, scale, Wq1, bq1, Wq2, bq2, gq, bq_ln,
                 Wk1, bk1, Wk2, bk2, gk, bk_ln, Ws, bs):
    f32 = np.float32
    ns = (scale.astype(f32) @ Ws.astype(f32) + bs.astype(f32))[0]
    qscale = f32(ns) / f32(np.sqrt(f32(NE)))

    def aug(w, b):
        return np.ascontiguousarray(
            np.concatenate([w.astype(f32), b.astype(f32)[None, :]], axis=0))

    def centered(w2a, g):
        # fold LN centering + gain into the second layer: rows centered
        # over the output dim, then scaled per-output by g
        c = w2a - w2a.mean(axis=1, keepdims=True)
        return np.ascontiguousarray((c * g[None, :]).astype(f32))

    def varmat(g):
        # varB = vmat.T @ (t0*g)^2 with vmat[d',d] = 1/(64*g[d']^2)
        col = (1.0 / (NE * g.astype(np.float64) ** 2)).astype(f32)
        return np.ascontiguousarray(np.repeat(col[:, None], NE, axis=1))

    gq_s = gq.astype(f32) * qscale
    lncoef = np.ascontiguousarray(np.stack(
        [bk_ln.astype(f32), bq_ln.astype(f32) * qscale], axis=1
    ).astype(f32))

    rays2 = np.ascontiguousarray(rays.reshape(N, NPOS).astype(f32))
    common = {
        "rays_all": rays2,
        "feats": np.ascontiguousarray(features.reshape(N, NE).astype(f32)),
        "id128": np.eye(128, dtype=f32),
        "lncoef": lncoef,
        "wk1a": aug(Wk1, bk1),
        "wk2c": centered(aug(Wk2, bk2), gk.astype(f32)),
        "wq1a": aug(Wq1, bq1),
        "wq2c": centered(aug(Wq2, bq2), gq_s),
        "vark": varmat(gk.astype(f32)),
        "varq": varmat(gq_s),
    }
    in_maps = []
    for c in range(N_CORES):
        m = dict(common)
        m["rays_sh"] = np.ascontiguousarray(rays2[c * SHARD : (c + 1) * SHARD])
        in_maps.append(m)
    return in_maps


def kernel(**inputs):
    if "nc" not in _CACHE:
        _CACHE["nc"] = _build_program()
    nc = _CACHE["nc"]
    in_maps = _prep_inputs(**inputs)
    res = run_bass_kernel_spmd(nc, in_maps, core_ids=list(range(N_CORES)))
    attn_w = np.concatenate([res.results[c]["attn_w"] for c in range(N_CORES)], axis=0)
    attn_o = np.concatenate([res.results[c]["attn_o"] for c in range(N_CORES)], axis=0)
    seq, h, w = 8, 32, 32
    return attn_o.reshape(seq, h, w, NE), attn_w
